# revision 4
# baseline (speedup 1.0000x reference)
"""Trainium2 Bass kernel for CustomAttentionClassifier.

Model (see reference): x = emb[ids] + pe; Q/K/V = x@W + b;
attn = softmax(QK^T/16); pooled = mean_s(attn @ V); logits = relu(pooled@Wc1+bc1)@Wc2+bc2.

Sharding: data-parallel over batch, B=64 -> 8 cores x 8 batches.

v2 restructuring (vs the gather-based v1):
- The embedding lookup + transpose happens on HOST: each core receives
  e^T per batch ([128, 2, S] bf16), so the device never touches the 15.6MB
  table and the pathological SWDGE transposed gather is gone.
- Host precomputes pQ = pe@Wq+bq (fp64) etc., so the device only adds the
  position-independent parts: Q^T = Wq^T e^T + pQ^T.
- mean-pool commutes with attn@V:  pooled = (mean_s attn) @ V, so the
  whole [S,S]x[S,D] context matmul is replaced by per-batch attention
  column means (abar). Scores are computed s-on-partitions; row sums come
  from a free-dim DVE reduction; abar^T = sum_s (1/rowsum_s) exp[s,:] is
  a PE matmul with a block-diagonal masked lhsT that accumulates all 8
  batches into one [8, 512] PSUM tile.
- The 1/S of the mean is folded into Wc1 on host; pe@Wv+bv is folded into
  the pooled matmul accumulation (extra lhsT terms), so V = e@Wv only.
"""

import numpy as np
import ml_dtypes

import concourse.bass as bass
import concourse.tile as tile
from concourse import bacc, mybir
from concourse.bass_utils import run_bass_kernel_spmd

V, D, S, B = 30522, 256, 512, 64
HID, NCLS = 128, 16
NCORES = 8
BL = B // NCORES          # 8 batches per core
SCH = S // 128            # 4 s/t chunks per batch

f32 = mybir.dt.float32
bf16 = mybir.dt.bfloat16

# knobs
import os as _os
STAGE = int(_os.environ.get("STAGE", "7"))  # debug truncation: 7=full


def _pos_encoding():
    pos = np.arange(S)[:, None].astype(np.float64)
    div = np.exp(np.arange(0, D, 2).astype(np.float64) * (-np.log(10000.0) / D))
    pe = np.zeros((S, D), dtype=np.float64)
    pe[:, 0::2] = np.sin(pos * div)
    pe[:, 1::2] = np.cos(pos * div)
    # match the reference, which builds pe in float32
    return pe.astype(np.float32)


def build_module():
    nc = bacc.Bacc("TRN2", target_bir_lowering=False, debug=False)

    et_d = [
        nc.dram_tensor(f"et{n}", [128, 2, S], bf16, kind="ExternalInput")
        for n in range(BL)
    ]
    wq_d = nc.dram_tensor("wq16", [128, 2, D], bf16, kind="ExternalInput")
    wk_d = nc.dram_tensor("wk16", [128, 2, D], bf16, kind="ExternalInput")
    wv_d = nc.dram_tensor("wv16", [128, 2, D], bf16, kind="ExternalInput")
    pqt_d = nc.dram_tensor("pqt", [128, 2, S], f32, kind="ExternalInput")
    pkt_d = nc.dram_tensor("pkt", [128, 2, S], f32, kind="ExternalInput")
    pv_d = nc.dram_tensor("pv16", [128, SCH, D], bf16, kind="ExternalInput")
    wc1_d = nc.dram_tensor("wc1", [128, 2, HID], f32, kind="ExternalInput")
    bc1_d = nc.dram_tensor("bc1c", [128, 1], f32, kind="ExternalInput")
    wc2_d = nc.dram_tensor("wc2", [128, NCLS], f32, kind="ExternalInput")
    bc2_d = nc.dram_tensor("bc2c", [16, 1], f32, kind="ExternalInput")
    out_d = nc.dram_tensor("lgt", [NCLS, BL], f32, kind="ExternalOutput")

    ADD = mybir.AluOpType.add
    EXP = mybir.ActivationFunctionType.Exp
    RELU = mybir.ActivationFunctionType.Relu
    AX = mybir.AxisListType.X

    with tile.TileContext(nc) as tc:
        with (
            tc.tile_pool(name="const", bufs=1) as cp,
            tc.tile_pool(name="work", bufs=3) as wp,
            tc.tile_pool(name="psA", bufs=2, space="PSUM") as psA,
            tc.tile_pool(name="psB", bufs=3, space="PSUM") as psB,
            tc.tile_pool(name="psC", bufs=1, space="PSUM") as psC,
        ):
            wq_s = cp.tile([128, 2, D], bf16, tag="wq")
            wk_s = cp.tile([128, 2, D], bf16, tag="wk")
            wv_s = cp.tile([128, 2, D], bf16, tag="wv")
            pqt_s = cp.tile([128, 2, S], f32, tag="pqt")
            pkt_s = cp.tile([128, 2, S], f32, tag="pkt")
            pv_s = cp.tile([128, SCH, D], bf16, tag="pv")
            wc1_s = cp.tile([128, 2, HID], f32, tag="wc1")
            bc1_s = cp.tile([128, 1], f32, tag="bc1")
            wc2_s = cp.tile([128, NCLS], f32, tag="wc2")
            bc2_s = cp.tile([16, 1], f32, tag="bc2")

            eTs = [
                cp.tile([128, 2, S], bf16, tag=f"eT{n}", name=f"eT{n}")
                for n in range(BL)
            ]
            qT = cp.tile([128, 2, BL * S], bf16, tag="qT")
            kT = cp.tile([128, 2, BL * S], bf16, tag="kT")
            vS = cp.tile([128, BL * SCH, D], bf16, tag="vS")
            rsum = cp.tile([128, BL, SCH], f32, tag="rsum")
            rc32 = cp.tile([128, BL, SCH], f32, tag="rc32")
            # block-diagonal masked lhsT for the abar matmuls:
            # rrbM[p, sc, n, col] = (col==n) * 1/rowsum_n[sc*128+p]
            rrbM = cp.tile([128, SCH, BL, BL], bf16, tag="rrbM")
            attnRows = cp.tile([32, S], bf16, tag="attnRows")
            attnT = cp.tile([128, SCH, 32], bf16, tag="attnT")
            pooledT = cp.tile([128, 2, BL], f32, tag="pooledT")
            hT = cp.tile([128, BL], f32, tag="hT")
            lgT = cp.tile([16, BL], f32, tag="lgT")

            nc.sync.dma_start(wq_s[:], wq_d.ap())
            nc.sync.dma_start(wk_s[:], wk_d.ap())
            nc.sync.dma_start(wv_s[:], wv_d.ap())
            nc.sync.dma_start(pqt_s[:], pqt_d.ap())
            nc.sync.dma_start(pkt_s[:], pkt_d.ap())
            nc.sync.dma_start(pv_s[:], pv_d.ap())
            nc.sync.dma_start(wc1_s[:], wc1_d.ap())
            nc.sync.dma_start(bc1_s[:], bc1_d.ap())
            nc.sync.dma_start(wc2_s[:], wc2_d.ap())
            nc.sync.dma_start(bc2_s[:], bc2_d.ap())
            for n in range(BL):
                nc.sync.dma_start(eTs[n][:], et_d[n].ap())

            nc.vector.memset(rrbM[:], 0.0)
            nc.vector.memset(attnRows[:], 0.0)
            if STAGE < 7:
                nc.vector.memset(lgT[:], 0.0)

            psAB = psC.tile([128, S], f32, tag="AB")  # rows 0:BL used
            expTiles = [None] * BL

            def emit_abar(n):
                # abar row n (x S): sum_s exp[s, t] / rowsum[s], accumulated
                # into the shared psAB via the masked lhsT. One long
                # accumulation group across all batches.
                for sc in range(SCH):
                    nc.tensor.matmul(
                        psAB[0:BL, :],
                        lhsT=rrbM[:, sc, n, :],
                        rhs=expTiles[n][:, sc, :],
                        start=(n == 0 and sc == 0),
                        stop=(n == BL - 1 and sc == SCH - 1),
                        skip_group_check=True,
                    )

            for n in range(BL if STAGE >= 1 else 0):
                # ---- Q^T, K^T for batch n ----
                for w_s, pT_s, oT in ((wq_s, pqt_s, qT), (wk_s, pkt_s, kT)):
                    ps = psA.tile([128, 2, S], f32, tag="A")
                    for m in range(2):
                        for k in range(2):
                            nc.tensor.matmul(
                                ps[:, m, :],
                                lhsT=w_s[:, k, m * 128:(m + 1) * 128],
                                rhs=eTs[n][:, k, :],
                                start=(k == 0),
                                stop=(k == 1),
                            )
                    nc.vector.tensor_tensor(
                        out=oT[:, :, n * S:(n + 1) * S],
                        in0=ps[:],
                        in1=pT_s[:],
                        op=ADD,
                    )

                # ---- V = e @ Wv for batch n (pe/bv part folded into pooled) ----
                if STAGE >= 2:
                    for sc in range(SCH):
                        psv = psB.tile([128, D], f32, tag="B")
                        for k in range(2):
                            nc.tensor.matmul(
                                psv[:],
                                lhsT=eTs[n][:, k, sc * 128:(sc + 1) * 128],
                                rhs=wv_s[:, k, :],
                                start=(k == 0),
                                stop=(k == 1),
                            )
                        nc.scalar.copy(out=vS[:, n * SCH + sc, :], in_=psv[:])

                # ---- scores (s on partitions) + exp + rowsums ----
                if STAGE >= 3:
                    expT = wp.tile([128, SCH, S], bf16, tag="expT")
                    expTiles[n] = expT
                    for w in range(2):
                        ps = psA.tile([128, 2, S], f32, tag="A")
                        for i in range(2):
                            sc = 2 * w + i
                            for m in range(2):
                                nc.tensor.matmul(
                                    ps[:, i, :],
                                    lhsT=qT[:, m, n * S + sc * 128: n * S + (sc + 1) * 128],
                                    rhs=kT[:, m, n * S:(n + 1) * S],
                                    start=(m == 0),
                                    stop=(m == 1),
                                )
                        nc.scalar.activation(
                            out=expT[:, 2 * w:2 * w + 2, :],
                            in_=ps[:],
                            func=EXP,
                            scale=1.0 / 16.0,
                        )
                        for i in range(2):
                            sc = 2 * w + i
                            nc.vector.tensor_reduce(
                                out=rsum[:, n, sc:sc + 1],
                                in_=expT[:, sc, :],
                                axis=AX,
                                op=ADD,
                            )
                    nc.vector.reciprocal(out=rc32[:, n, :], in_=rsum[:, n, :])
                    nc.vector.tensor_copy(out=rrbM[:, :, n, n], in_=rc32[:, n, :])

                    # software-pipelined by one batch: emit abar(n-1) here so
                    # PE never stalls waiting on this batch's exp/rowsum.
                    if STAGE >= 4 and n > 0:
                        emit_abar(n - 1)

            if STAGE >= 4:
                emit_abar(BL - 1)
                nc.scalar.copy(out=attnRows[0:BL, :], in_=psAB[0:BL, :])

            if STAGE >= 5:
                # transpose abar rows -> columns: attnT[p, sc, b] = abar_b[sc*128+p]
                for g in range(16):
                    sc, j = g // 4, g % 4
                    nc.vector.transpose(
                        out=attnT[j * 32:(j + 1) * 32, sc, :],
                        in_=attnRows[0:32, g * 32:(g + 1) * 32],
                    )

            if STAGE >= 6:
                # pooled^T (x S, folded into wc1): for each d-chunk, accumulate
                # all 8 batches into one [128, BL] psum (per-column groups).
                for dch in range(2):
                    psp = psB.tile([128, BL], f32, tag="B")
                    for n in range(BL):
                        for tc in range(SCH):
                            nc.tensor.matmul(
                                psp[:, n:n + 1],
                                lhsT=vS[:, n * SCH + tc, dch * 128:(dch + 1) * 128],
                                rhs=attnT[:, tc, n:n + 1],
                                start=(tc == 0),
                                stop=False,
                                skip_group_check=True,
                            )
                            nc.tensor.matmul(
                                psp[:, n:n + 1],
                                lhsT=pv_s[:, tc, dch * 128:(dch + 1) * 128],
                                rhs=attnT[:, tc, n:n + 1],
                                start=False,
                                stop=(tc == SCH - 1),
                                skip_group_check=True,
                            )
                    nc.scalar.copy(out=pooledT[:, dch, :], in_=psp[:, 0:BL])

            # ---- classifier ----
            if STAGE >= 7:
                hps = psB.tile([128, BL], f32, tag="B")
                for k in range(2):
                    nc.tensor.matmul(
                        hps[:, 0:BL],
                        lhsT=wc1_s[:, k, :],
                        rhs=pooledT[:, k, :],
                        start=(k == 0),
                        stop=(k == 1),
                    )
                nc.scalar.activation(
                    out=hT[:], in_=hps[:, 0:BL], func=RELU, bias=bc1_s[:]
                )

                lps = psB.tile([128, BL], f32, tag="B")
                nc.tensor.matmul(
                    lps[0:NCLS, 0:BL], lhsT=wc2_s[:], rhs=hT[:], start=True, stop=True
                )
                nc.vector.tensor_tensor(
                    out=lgT[:],
                    in0=lps[0:NCLS, 0:BL],
                    in1=bc2_s[:].to_broadcast([NCLS, BL]),
                    op=ADD,
                )
            nc.sync.dma_start(out_d.ap(), lgT[:])

    nc.compile()
    return nc


def prepare_in_maps(input_ids, emb, Wq, bq, Wk, bk, Wv, bv, Wc1, bc1, Wc2, bc2):
    pe = _pos_encoding().astype(np.float64)
    pQ = (pe @ Wq.astype(np.float64) + bq.astype(np.float64)).astype(np.float32)
    pK = (pe @ Wk.astype(np.float64) + bk.astype(np.float64)).astype(np.float32)
    pV = (pe @ Wv.astype(np.float64) + bv.astype(np.float64)).astype(np.float32)

    emb16 = emb.astype(ml_dtypes.bfloat16)

    def chunk_w(w):  # [D, D] -> [128, 2, D] bf16 with [p,k,j] = w[k*128+p, j]
        return np.ascontiguousarray(
            w.reshape(2, 128, D).transpose(1, 0, 2).astype(ml_dtypes.bfloat16)
        )

    wq16 = chunk_w(Wq)
    wk16 = chunk_w(Wk)
    wv16 = chunk_w(Wv)

    def chunk_pT(p):  # [S, D] -> [128, 2, S] f32 with [p_,m,s] = p[s, m*128+p_]
        return np.ascontiguousarray(p.T.reshape(2, 128, S).transpose(1, 0, 2)).astype(
            np.float32
        )

    pqt = chunk_pT(pQ)
    pkt = chunk_pT(pK)
    # pv16[p, sc, d] = pV[sc*128+p, d]
    pv16 = np.ascontiguousarray(
        pV.reshape(SCH, 128, D).transpose(1, 0, 2)
    ).astype(ml_dtypes.bfloat16)
    # 1/S of the mean pooling is folded in here
    wc1 = np.ascontiguousarray(
        (Wc1 / np.float32(S)).reshape(2, 128, HID).transpose(1, 0, 2).astype(np.float32)
    )
    bc1c = np.ascontiguousarray(bc1.reshape(HID, 1).astype(np.float32))
    wc2 = np.ascontiguousarray(Wc2.astype(np.float32))
    bc2c = np.ascontiguousarray(bc2.reshape(NCLS, 1).astype(np.float32))

    in_maps = []
    for c in range(NCORES):
        m = dict(
            wq16=wq16,
            wk16=wk16,
            wv16=wv16,
            pqt=pqt,
            pkt=pkt,
            pv16=pv16,
            wc1=wc1,
            bc1c=bc1c,
            wc2=wc2,
            bc2c=bc2c,
        )
        for n in range(BL):
            e = emb16[input_ids[c * BL + n]]  # [S, D] bf16, host-side gather
            # et[p, k, s] = e[s, k*128+p]
            m[f"et{n}"] = np.ascontiguousarray(
                e.T.reshape(2, 128, S).transpose(1, 0, 2)
            )
        in_maps.append(m)
    return in_maps


_NC_CACHE = {}


def kernel(**inputs):
    inputs = {k: np.asarray(v) for k, v in inputs.items()}
    if "nc" not in _NC_CACHE:
        _NC_CACHE["nc"] = build_module()
    nc = _NC_CACHE["nc"]
    in_maps = prepare_in_maps(**inputs)
    res = run_bass_kernel_spmd(nc, in_maps, core_ids=list(range(NCORES)))
    out = np.empty((B, NCLS), dtype=np.float32)
    for c in range(NCORES):
        out[c * BL:(c + 1) * BL] = res.results[c]["lgt"].T
    return out


# revision 6
# speedup vs baseline: 1.1170x; 1.1170x over previous
"""Trainium2 Bass kernel for CustomAttentionClassifier.

Model (see reference): x = emb[ids] + pe; Q/K/V = x@W + b;
attn = softmax(QK^T/16); pooled = mean_s(attn @ V); logits = relu(pooled@Wc1+bc1)@Wc2+bc2.

Sharding: data-parallel over batch, B=64 -> 8 cores x 8 batches.

v2 restructuring (vs the gather-based v1):
- The embedding lookup + transpose happens on HOST: each core receives
  e^T per batch ([128, 2, S] bf16), so the device never touches the 15.6MB
  table and the pathological SWDGE transposed gather is gone.
- Host precomputes pQ = pe@Wq+bq (fp64) etc., so the device only adds the
  position-independent parts: Q^T = Wq^T e^T + pQ^T.
- mean-pool commutes with attn@V:  pooled = (mean_s attn) @ V, so the
  whole [S,S]x[S,D] context matmul is replaced by per-batch attention
  column means (abar). Scores are computed s-on-partitions; row sums come
  from a free-dim DVE reduction; abar^T = sum_s (1/rowsum_s) exp[s,:] is
  a PE matmul with a block-diagonal masked lhsT that accumulates all 8
  batches into one [8, 512] PSUM tile.
- The 1/S of the mean is folded into Wc1 on host; pe@Wv+bv is folded into
  the pooled matmul accumulation (extra lhsT terms), so V = e@Wv only.
"""

import numpy as np
import ml_dtypes

import concourse.bass as bass
import concourse.tile as tile
from concourse import bacc, mybir
from concourse.bass_utils import run_bass_kernel_spmd

V, D, S, B = 30522, 256, 512, 64
HID, NCLS = 128, 16
NCORES = 8
BL = B // NCORES          # 8 batches per core
SCH = S // 128            # 4 s/t chunks per batch

f32 = mybir.dt.float32
bf16 = mybir.dt.bfloat16

# knobs
import os as _os
STAGE = int(_os.environ.get("STAGE", "7"))  # debug truncation: 7=full


def _pos_encoding():
    pos = np.arange(S)[:, None].astype(np.float64)
    div = np.exp(np.arange(0, D, 2).astype(np.float64) * (-np.log(10000.0) / D))
    pe = np.zeros((S, D), dtype=np.float64)
    pe[:, 0::2] = np.sin(pos * div)
    pe[:, 1::2] = np.cos(pos * div)
    # match the reference, which builds pe in float32
    return pe.astype(np.float32)


def build_module():
    nc = bacc.Bacc("TRN2", target_bir_lowering=False, debug=False)

    et_d = [
        nc.dram_tensor(f"et{n}", [128, 2, S], bf16, kind="ExternalInput")
        for n in range(BL)
    ]
    wq_d = nc.dram_tensor("wq16", [128, 2, D], bf16, kind="ExternalInput")
    wk_d = nc.dram_tensor("wk16", [128, 2, D], bf16, kind="ExternalInput")
    wv_d = nc.dram_tensor("wv16", [128, 2, D], bf16, kind="ExternalInput")
    pqt_d = nc.dram_tensor("pqt", [128, 2, S], bf16, kind="ExternalInput")
    pkt_d = nc.dram_tensor("pkt", [128, 2, S], bf16, kind="ExternalInput")
    pv_d = nc.dram_tensor("pv16", [128, SCH, D], bf16, kind="ExternalInput")
    wc1_d = nc.dram_tensor("wc1", [128, 2, HID], f32, kind="ExternalInput")
    bc1_d = nc.dram_tensor("bc1c", [128, 1], f32, kind="ExternalInput")
    wc2_d = nc.dram_tensor("wc2", [128, NCLS], f32, kind="ExternalInput")
    bc2_d = nc.dram_tensor("bc2c", [16, 1], f32, kind="ExternalInput")
    out_d = nc.dram_tensor("lgt", [NCLS, BL], f32, kind="ExternalOutput")

    ADD = mybir.AluOpType.add
    EXP = mybir.ActivationFunctionType.Exp
    RELU = mybir.ActivationFunctionType.Relu
    AX = mybir.AxisListType.X

    with tile.TileContext(nc) as tc:
        with (
            tc.tile_pool(name="const", bufs=1) as cp,
            tc.tile_pool(name="work", bufs=3) as wp,
            tc.tile_pool(name="psA", bufs=2, space="PSUM") as psA,
            tc.tile_pool(name="psB", bufs=3, space="PSUM") as psB,
            tc.tile_pool(name="psC", bufs=1, space="PSUM") as psC,
        ):
            wq_s = cp.tile([128, 2, D], bf16, tag="wq")
            wk_s = cp.tile([128, 2, D], bf16, tag="wk")
            wv_s = cp.tile([128, 2, D], bf16, tag="wv")
            pqt_s = cp.tile([128, 2, S], bf16, tag="pqt")
            pkt_s = cp.tile([128, 2, S], bf16, tag="pkt")
            pv_s = cp.tile([128, SCH, D], bf16, tag="pv")
            wc1_s = cp.tile([128, 2, HID], f32, tag="wc1")
            bc1_s = cp.tile([128, 1], f32, tag="bc1")
            wc2_s = cp.tile([128, NCLS], f32, tag="wc2")
            bc2_s = cp.tile([16, 1], f32, tag="bc2")

            eTs = [
                cp.tile([128, 2, S], bf16, tag=f"eT{n}", name=f"eT{n}")
                for n in range(BL)
            ]
            qT = cp.tile([128, 2, BL * S], bf16, tag="qT")
            kT = cp.tile([128, 2, BL * S], bf16, tag="kT")
            vS = cp.tile([128, BL * SCH, D], bf16, tag="vS")
            rsum = cp.tile([128, BL, SCH], f32, tag="rsum")
            rc32 = cp.tile([128, BL, SCH], f32, tag="rc32")
            # block-diagonal masked lhsT for the abar matmuls:
            # rrbM[p, sc, n, col] = (col==n) * 1/rowsum_n[sc*128+p]
            rrbM = cp.tile([128, SCH, BL, BL], bf16, tag="rrbM")
            attnRows = cp.tile([32, S], bf16, tag="attnRows")
            attnT = cp.tile([128, SCH, 32], bf16, tag="attnT")
            pooledT = cp.tile([128, 2, BL], f32, tag="pooledT")
            hT = cp.tile([128, BL], f32, tag="hT")
            lgT = cp.tile([16, BL], f32, tag="lgT")

            nc.sync.dma_start(eTs[0][:], et_d[0].ap())
            nc.sync.dma_start(wq_s[:], wq_d.ap())
            nc.sync.dma_start(wk_s[:], wk_d.ap())
            nc.sync.dma_start(pqt_s[:], pqt_d.ap())
            nc.sync.dma_start(pkt_s[:], pkt_d.ap())
            nc.sync.dma_start(wv_s[:], wv_d.ap())
            nc.sync.dma_start(eTs[1][:], et_d[1].ap())
            nc.sync.dma_start(eTs[2][:], et_d[2].ap())
            nc.sync.dma_start(pv_s[:], pv_d.ap())
            nc.sync.dma_start(wc1_s[:], wc1_d.ap())
            nc.sync.dma_start(bc1_s[:], bc1_d.ap())
            nc.sync.dma_start(wc2_s[:], wc2_d.ap())
            nc.sync.dma_start(bc2_s[:], bc2_d.ap())
            for n in range(3, BL):
                nc.sync.dma_start(eTs[n][:], et_d[n].ap())

            nc.gpsimd.memset(rrbM[:], 0.0)
            nc.gpsimd.memset(attnRows[:], 0.0)
            if STAGE < 7:
                nc.vector.memset(lgT[:], 0.0)

            psAB = psC.tile([128, S], f32, tag="AB")  # rows 0:BL used
            expTiles = [None] * BL

            def emit_abar(n):
                # abar row n (x S): sum_s exp[s, t] / rowsum[s], accumulated
                # into the shared psAB via the masked lhsT. One long
                # accumulation group across all batches.
                for sc in range(SCH):
                    nc.tensor.matmul(
                        psAB[0:BL, :],
                        lhsT=rrbM[:, sc, n, :],
                        rhs=expTiles[n][:, sc, :],
                        start=(n == 0 and sc == 0),
                        stop=(n == BL - 1 and sc == SCH - 1),
                        skip_group_check=True,
                    )

            for n in range(BL if STAGE >= 1 else 0):
                # ---- Q^T, K^T for batch n ----
                for w_s, pT_s, oT in ((wq_s, pqt_s, qT), (wk_s, pkt_s, kT)):
                    ps = psA.tile([128, 2, S], f32, tag="A")
                    for m in range(2):
                        for k in range(2):
                            nc.tensor.matmul(
                                ps[:, m, :],
                                lhsT=w_s[:, k, m * 128:(m + 1) * 128],
                                rhs=eTs[n][:, k, :],
                                start=(k == 0),
                                stop=(k == 1),
                            )
                    nc.vector.tensor_tensor(
                        out=oT[:, :, n * S:(n + 1) * S],
                        in0=ps[:],
                        in1=pT_s[:],
                        op=ADD,
                    )

                # ---- V = e @ Wv for batch n (pe/bv part folded into pooled) ----
                if STAGE >= 2:
                    for h in range(2):
                        psv = psB.tile([128, 2, D], f32, tag="B")
                        for j in range(2):
                            sc = 2 * h + j
                            for k in range(2):
                                nc.tensor.matmul(
                                    psv[:, j, :],
                                    lhsT=eTs[n][:, k, sc * 128:(sc + 1) * 128],
                                    rhs=wv_s[:, k, :],
                                    start=(k == 0),
                                    stop=(k == 1),
                                )
                        nc.vector.tensor_copy(
                            out=vS[:, n * SCH + 2 * h:n * SCH + 2 * h + 2, :],
                            in_=psv[:],
                        )

                # ---- scores (s on partitions) + exp + rowsums ----
                if STAGE >= 3:
                    expT = wp.tile([128, SCH, S], bf16, tag="expT")
                    expTiles[n] = expT
                    for w in range(2):
                        ps = psA.tile([128, 2, S], f32, tag="A")
                        for i in range(2):
                            sc = 2 * w + i
                            for m in range(2):
                                nc.tensor.matmul(
                                    ps[:, i, :],
                                    lhsT=qT[:, m, n * S + sc * 128: n * S + (sc + 1) * 128],
                                    rhs=kT[:, m, n * S:(n + 1) * S],
                                    start=(m == 0),
                                    stop=(m == 1),
                                )
                        for i in range(2):
                            sc = 2 * w + i
                            nc.scalar.activation(
                                out=expT[:, sc, :],
                                in_=ps[:, i, :],
                                func=EXP,
                                scale=1.0 / 16.0,
                                accum_out=rsum[:, n, sc:sc + 1],
                            )
                    nc.vector.reciprocal(out=rc32[:, n, :], in_=rsum[:, n, :])
                    nc.vector.tensor_copy(out=rrbM[:, :, n, n], in_=rc32[:, n, :])

                    # software-pipelined by one batch: emit abar(n-1) here so
                    # PE never stalls waiting on this batch's exp/rowsum.
                    if STAGE >= 4 and n > 0:
                        emit_abar(n - 1)

            if STAGE >= 4:
                emit_abar(BL - 1)
                nc.scalar.copy(out=attnRows[0:BL, :], in_=psAB[0:BL, :])

            if STAGE >= 5:
                # transpose abar rows -> columns: attnT[p, sc, b] = abar_b[sc*128+p]
                for g in range(16):
                    sc, j = g // 4, g % 4
                    nc.vector.transpose(
                        out=attnT[j * 32:(j + 1) * 32, sc, :],
                        in_=attnRows[0:32, g * 32:(g + 1) * 32],
                    )

            if STAGE >= 6:
                # pooled^T (x S, folded into wc1): for each d-chunk, accumulate
                # all 8 batches into one [128, BL] psum (per-column groups).
                for dch in range(2):
                    psp = psB.tile([128, BL], f32, tag="B")
                    for n in range(BL):
                        for tc in range(SCH):
                            nc.tensor.matmul(
                                psp[:, n:n + 1],
                                lhsT=vS[:, n * SCH + tc, dch * 128:(dch + 1) * 128],
                                rhs=attnT[:, tc, n:n + 1],
                                start=(tc == 0),
                                stop=False,
                                skip_group_check=True,
                            )
                            nc.tensor.matmul(
                                psp[:, n:n + 1],
                                lhsT=pv_s[:, tc, dch * 128:(dch + 1) * 128],
                                rhs=attnT[:, tc, n:n + 1],
                                start=False,
                                stop=(tc == SCH - 1),
                                skip_group_check=True,
                            )
                    nc.scalar.copy(out=pooledT[:, dch, :], in_=psp[:, 0:BL])

            # ---- classifier ----
            if STAGE >= 7:
                hps = psB.tile([128, BL], f32, tag="B")
                for k in range(2):
                    nc.tensor.matmul(
                        hps[:, 0:BL],
                        lhsT=wc1_s[:, k, :],
                        rhs=pooledT[:, k, :],
                        start=(k == 0),
                        stop=(k == 1),
                    )
                nc.scalar.activation(
                    out=hT[:], in_=hps[:, 0:BL], func=RELU, bias=bc1_s[:]
                )

                lps = psB.tile([128, BL], f32, tag="B")
                nc.tensor.matmul(
                    lps[0:NCLS, 0:BL], lhsT=wc2_s[:], rhs=hT[:], start=True, stop=True
                )
                nc.vector.tensor_tensor(
                    out=lgT[:],
                    in0=lps[0:NCLS, 0:BL],
                    in1=bc2_s[:].to_broadcast([NCLS, BL]),
                    op=ADD,
                )
            nc.sync.dma_start(out_d.ap(), lgT[:])

    nc.compile()
    return nc


def prepare_in_maps(input_ids, emb, Wq, bq, Wk, bk, Wv, bv, Wc1, bc1, Wc2, bc2):
    pe = _pos_encoding().astype(np.float64)
    pQ = (pe @ Wq.astype(np.float64) + bq.astype(np.float64)).astype(np.float32)
    pK = (pe @ Wk.astype(np.float64) + bk.astype(np.float64)).astype(np.float32)
    pV = (pe @ Wv.astype(np.float64) + bv.astype(np.float64)).astype(np.float32)

    emb16 = emb.astype(ml_dtypes.bfloat16)

    def chunk_w(w):  # [D, D] -> [128, 2, D] bf16 with [p,k,j] = w[k*128+p, j]
        return np.ascontiguousarray(
            w.reshape(2, 128, D).transpose(1, 0, 2).astype(ml_dtypes.bfloat16)
        )

    wq16 = chunk_w(Wq)
    wk16 = chunk_w(Wk)
    wv16 = chunk_w(Wv)

    def chunk_pT(p):  # [S, D] -> [128, 2, S] f32 with [p_,m,s] = p[s, m*128+p_]
        return np.ascontiguousarray(p.T.reshape(2, 128, S).transpose(1, 0, 2)).astype(
            np.float32
        )

    pqt = chunk_pT(pQ).astype(ml_dtypes.bfloat16)
    pkt = chunk_pT(pK).astype(ml_dtypes.bfloat16)
    # pv16[p, sc, d] = pV[sc*128+p, d]
    pv16 = np.ascontiguousarray(
        pV.reshape(SCH, 128, D).transpose(1, 0, 2)
    ).astype(ml_dtypes.bfloat16)
    # 1/S of the mean pooling is folded in here
    wc1 = np.ascontiguousarray(
        (Wc1 / np.float32(S)).reshape(2, 128, HID).transpose(1, 0, 2).astype(np.float32)
    )
    bc1c = np.ascontiguousarray(bc1.reshape(HID, 1).astype(np.float32))
    wc2 = np.ascontiguousarray(Wc2.astype(np.float32))
    bc2c = np.ascontiguousarray(bc2.reshape(NCLS, 1).astype(np.float32))

    in_maps = []
    for c in range(NCORES):
        m = dict(
            wq16=wq16,
            wk16=wk16,
            wv16=wv16,
            pqt=pqt,
            pkt=pkt,
            pv16=pv16,
            wc1=wc1,
            bc1c=bc1c,
            wc2=wc2,
            bc2c=bc2c,
        )
        for n in range(BL):
            e = emb16[input_ids[c * BL + n]]  # [S, D] bf16, host-side gather
            # et[p, k, s] = e[s, k*128+p]
            m[f"et{n}"] = np.ascontiguousarray(
                e.T.reshape(2, 128, S).transpose(1, 0, 2)
            )
        in_maps.append(m)
    return in_maps


_NC_CACHE = {}


def kernel(**inputs):
    inputs = {k: np.asarray(v) for k, v in inputs.items()}
    if "nc" not in _NC_CACHE:
        _NC_CACHE["nc"] = build_module()
    nc = _NC_CACHE["nc"]
    in_maps = prepare_in_maps(**inputs)
    res = run_bass_kernel_spmd(nc, in_maps, core_ids=list(range(NCORES)))
    out = np.empty((B, NCLS), dtype=np.float32)
    for c in range(NCORES):
        out[c * BL:(c + 1) * BL] = res.results[c]["lgt"].T
    return out


# revision 8
# speedup vs baseline: 1.2974x; 1.1614x over previous
"""Trainium2 Bass kernel for CustomAttentionClassifier.

Model (see reference): x = emb[ids] + pe; Q/K/V = x@W + b;
attn = softmax(QK^T/16); pooled = mean_s(attn @ V); logits = relu(pooled@Wc1+bc1)@Wc2+bc2.

Sharding: data-parallel over batch, B=64 -> 8 cores x 8 batches.

v2 restructuring (vs the gather-based v1):
- The embedding lookup + transpose happens on HOST: each core receives
  e^T per batch ([128, 2, S] bf16), so the device never touches the 15.6MB
  table and the pathological SWDGE transposed gather is gone.
- Host precomputes pQ = pe@Wq+bq (fp64) etc., so the device only adds the
  position-independent parts: Q^T = Wq^T e^T + pQ^T.
- mean-pool commutes with attn@V:  pooled = (mean_s attn) @ V, so the
  whole [S,S]x[S,D] context matmul is replaced by per-batch attention
  column means (abar). Scores are computed s-on-partitions; row sums come
  from a free-dim DVE reduction; abar^T = sum_s (1/rowsum_s) exp[s,:] is
  a PE matmul with a block-diagonal masked lhsT that accumulates all 8
  batches into one [8, 512] PSUM tile.
- The 1/S of the mean is folded into Wc1 on host; pe@Wv+bv is folded into
  the pooled matmul accumulation (extra lhsT terms), so V = e@Wv only.
"""

import numpy as np
import ml_dtypes

import concourse.bass as bass
import concourse.tile as tile
from concourse import bacc, mybir
from concourse.bass_utils import run_bass_kernel_spmd

V, D, S, B = 30522, 256, 512, 64
HID, NCLS = 128, 16
NCORES = 8
BL = B // NCORES          # 8 batches per core
SCH = S // 128            # 4 s/t chunks per batch

f32 = mybir.dt.float32
bf16 = mybir.dt.bfloat16

# knobs
import os as _os
STAGE = int(_os.environ.get("STAGE", "7"))  # debug truncation: 7=full


def _pos_encoding():
    pos = np.arange(S)[:, None].astype(np.float64)
    div = np.exp(np.arange(0, D, 2).astype(np.float64) * (-np.log(10000.0) / D))
    pe = np.zeros((S, D), dtype=np.float64)
    pe[:, 0::2] = np.sin(pos * div)
    pe[:, 1::2] = np.cos(pos * div)
    # match the reference, which builds pe in float32
    return pe.astype(np.float32)


def build_module():
    nc = bacc.Bacc("TRN2", target_bir_lowering=False, debug=False)

    et_d = [
        nc.dram_tensor(f"et{n}", [128, 2, S], bf16, kind="ExternalInput")
        for n in range(BL)
    ]
    wq_d = nc.dram_tensor("wq16", [128, 2, D], bf16, kind="ExternalInput")
    wk_d = nc.dram_tensor("wk16", [128, 2, D], bf16, kind="ExternalInput")
    wv_d = nc.dram_tensor("wv16", [128, 2, D], bf16, kind="ExternalInput")
    pqt_d = nc.dram_tensor("pqt", [128, 2, S], bf16, kind="ExternalInput")
    pkt_d = nc.dram_tensor("pkt", [128, 2, S], bf16, kind="ExternalInput")
    pv_d = nc.dram_tensor("pv16", [128, SCH, D], bf16, kind="ExternalInput")
    wc1_d = nc.dram_tensor("wc1", [128, 2, HID], f32, kind="ExternalInput")
    bc1_d = nc.dram_tensor("bc1c", [128, 1], f32, kind="ExternalInput")
    wc2_d = nc.dram_tensor("wc2", [128, NCLS], f32, kind="ExternalInput")
    bc2_d = nc.dram_tensor("bc2c", [16, 1], f32, kind="ExternalInput")
    out_d = nc.dram_tensor("lgt", [NCLS, BL], f32, kind="ExternalOutput")

    ADD = mybir.AluOpType.add
    EXP = mybir.ActivationFunctionType.Exp
    RELU = mybir.ActivationFunctionType.Relu
    AX = mybir.AxisListType.X

    with tile.TileContext(nc) as tc:
        with (
            tc.tile_pool(name="const", bufs=1) as cp,
            tc.tile_pool(name="work", bufs=3) as wp,
            tc.tile_pool(name="psA", bufs=2, space="PSUM") as psA,
            tc.tile_pool(name="psB", bufs=3, space="PSUM") as psB,
            tc.tile_pool(name="psC", bufs=1, space="PSUM") as psC,
        ):
            wq_s = cp.tile([128, 2, D], bf16, tag="wq")
            wk_s = cp.tile([128, 2, D], bf16, tag="wk")
            wv_s = cp.tile([128, 2, D], bf16, tag="wv")
            pqt_s = cp.tile([128, 2, S], bf16, tag="pqt")
            pkt_s = cp.tile([128, 2, S], bf16, tag="pkt")
            pv_s = cp.tile([128, SCH, D], bf16, tag="pv")
            wc1_s = cp.tile([128, 2, HID], f32, tag="wc1")
            bc1_s = cp.tile([128, 1], f32, tag="bc1")
            wc2_s = cp.tile([128, NCLS], f32, tag="wc2")
            bc2_s = cp.tile([16, 1], f32, tag="bc2")

            eTs = [
                cp.tile([128, 2, S], bf16, tag=f"eT{n}", name=f"eT{n}")
                for n in range(BL)
            ]
            qT = cp.tile([128, 2, BL * S], bf16, tag="qT")
            kT = cp.tile([128, 2, BL * S], bf16, tag="kT")
            vS = cp.tile([128, BL * SCH, D], bf16, tag="vS")
            rsum = cp.tile([128, BL, SCH], f32, tag="rsum")
            rc32 = cp.tile([128, BL, SCH], f32, tag="rc32")
            # block-diagonal masked lhsT for the abar matmuls:
            # rrbM[p, sc, n, col] = (col==n) * 1/rowsum_n[sc*128+p]
            rrbM = cp.tile([128, SCH, BL, BL], bf16, tag="rrbM")
            attnRows = cp.tile([32, S], bf16, tag="attnRows")
            attnT = cp.tile([128, SCH, 32], bf16, tag="attnT")
            pooledT = cp.tile([128, 2, BL], f32, tag="pooledT")
            hT = cp.tile([128, BL], f32, tag="hT")
            lgT = cp.tile([16, BL], f32, tag="lgT")

            nc.sync.dma_start(eTs[0][:], et_d[0].ap())
            nc.sync.dma_start(wq_s[:], wq_d.ap())
            nc.sync.dma_start(wk_s[:], wk_d.ap())
            nc.sync.dma_start(pqt_s[:], pqt_d.ap())
            nc.sync.dma_start(pkt_s[:], pkt_d.ap())
            nc.sync.dma_start(wv_s[:], wv_d.ap())
            nc.sync.dma_start(eTs[1][:], et_d[1].ap())
            nc.sync.dma_start(eTs[2][:], et_d[2].ap())
            nc.sync.dma_start(pv_s[:], pv_d.ap())
            nc.sync.dma_start(wc1_s[:], wc1_d.ap())
            nc.sync.dma_start(bc1_s[:], bc1_d.ap())
            nc.sync.dma_start(wc2_s[:], wc2_d.ap())
            nc.sync.dma_start(bc2_s[:], bc2_d.ap())
            for n in range(3, BL):
                nc.sync.dma_start(eTs[n][:], et_d[n].ap())

            nc.gpsimd.memset(rrbM[:], 0.0)
            nc.gpsimd.memset(attnRows[:], 0.0)
            if STAGE < 7:
                nc.vector.memset(lgT[:], 0.0)

            psAB = psC.tile([128, S], f32, tag="AB")  # rows 0:BL used
            expTiles = [None] * BL

            def emit_abar(n):
                # abar row n (x S): sum_s exp[s, t] / rowsum[s], accumulated
                # into the shared psAB via the masked lhsT. One long
                # accumulation group across all batches.
                for sc in range(SCH):
                    nc.tensor.matmul(
                        psAB[0:BL, :],
                        lhsT=rrbM[:, sc, n, :],
                        rhs=expTiles[n][:, sc, :],
                        start=(n == 0 and sc == 0),
                        stop=(n == BL - 1 and sc == SCH - 1),
                        skip_group_check=True,
                    )

            for n in range(BL if STAGE >= 1 else 0):
                # ---- Q^T, K^T for batch n ----
                for w_s, pT_s, oT in ((wq_s, pqt_s, qT), (wk_s, pkt_s, kT)):
                    for m in range(2):
                        ps = psB.tile([128, S], f32, tag="B")
                        for k in range(2):
                            nc.tensor.matmul(
                                ps[:],
                                lhsT=w_s[:, k, m * 128:(m + 1) * 128],
                                rhs=eTs[n][:, k, :],
                                start=(k == 0),
                                stop=(k == 1),
                            )
                        nc.vector.tensor_tensor(
                            out=oT[:, m, n * S:(n + 1) * S],
                            in0=ps[:],
                            in1=pT_s[:, m, :],
                            op=ADD,
                        )

                # ---- V = e @ Wv for batch n (pe/bv part folded into pooled) ----
                if STAGE >= 2:
                    for sc in range(SCH):
                        psv = psB.tile([128, D], f32, tag="B")
                        for k in range(2):
                            nc.tensor.matmul(
                                psv[:],
                                lhsT=eTs[n][:, k, sc * 128:(sc + 1) * 128],
                                rhs=wv_s[:, k, :],
                                start=(k == 0),
                                stop=(k == 1),
                            )
                        nc.vector.tensor_copy(out=vS[:, n * SCH + sc, :], in_=psv[:])

                # ---- scores (s on partitions) + exp + rowsums ----
                if STAGE >= 3:
                    expT = wp.tile([128, SCH, S], bf16, tag="expT")
                    expTiles[n] = expT
                    for w in range(2):
                        ps = psA.tile([128, 2, S], f32, tag="A")
                        for i in range(2):
                            sc = 2 * w + i
                            for m in range(2):
                                nc.tensor.matmul(
                                    ps[:, i, :],
                                    lhsT=qT[:, m, n * S + sc * 128: n * S + (sc + 1) * 128],
                                    rhs=kT[:, m, n * S:(n + 1) * S],
                                    start=(m == 0),
                                    stop=(m == 1),
                                )
                        for i in range(2):
                            sc = 2 * w + i
                            nc.scalar.activation(
                                out=expT[:, sc, :],
                                in_=ps[:, i, :],
                                func=EXP,
                                scale=1.0 / 16.0,
                                accum_out=rsum[:, n, sc:sc + 1],
                            )
                    nc.vector.reciprocal(out=rc32[:, n, :], in_=rsum[:, n, :])
                    nc.vector.tensor_copy(out=rrbM[:, :, n, n], in_=rc32[:, n, :])

                    # software-pipelined by one batch: emit abar(n-1) here so
                    # PE never stalls waiting on this batch's exp/rowsum.
                    if STAGE >= 4 and n > 0:
                        emit_abar(n - 1)

            if STAGE >= 4:
                emit_abar(BL - 1)
                nc.scalar.copy(out=attnRows[0:BL, :], in_=psAB[0:BL, :])

            if STAGE >= 5:
                # transpose abar rows -> columns: attnT[p, sc, b] = abar_b[sc*128+p]
                for g in range(16):
                    sc, j = g // 4, g % 4
                    nc.vector.transpose(
                        out=attnT[j * 32:(j + 1) * 32, sc, :],
                        in_=attnRows[0:32, g * 32:(g + 1) * 32],
                    )

            if STAGE >= 6:
                # pooled^T (x S, folded into wc1): for each d-chunk, accumulate
                # all 8 batches into one [128, BL] psum (per-column groups).
                for dch in range(2):
                    psp = psB.tile([128, BL], f32, tag="B")
                    for n in range(BL):
                        for tc in range(SCH):
                            nc.tensor.matmul(
                                psp[:, n:n + 1],
                                lhsT=vS[:, n * SCH + tc, dch * 128:(dch + 1) * 128],
                                rhs=attnT[:, tc, n:n + 1],
                                start=(tc == 0),
                                stop=False,
                                skip_group_check=True,
                            )
                            nc.tensor.matmul(
                                psp[:, n:n + 1],
                                lhsT=pv_s[:, tc, dch * 128:(dch + 1) * 128],
                                rhs=attnT[:, tc, n:n + 1],
                                start=False,
                                stop=(tc == SCH - 1),
                                skip_group_check=True,
                            )
                    nc.scalar.copy(out=pooledT[:, dch, :], in_=psp[:, 0:BL])

            # ---- classifier ----
            if STAGE >= 7:
                hps = psB.tile([128, BL], f32, tag="B")
                for k in range(2):
                    nc.tensor.matmul(
                        hps[:, 0:BL],
                        lhsT=wc1_s[:, k, :],
                        rhs=pooledT[:, k, :],
                        start=(k == 0),
                        stop=(k == 1),
                    )
                nc.scalar.activation(
                    out=hT[:], in_=hps[:, 0:BL], func=RELU, bias=bc1_s[:]
                )

                lps = psB.tile([128, BL], f32, tag="B")
                nc.tensor.matmul(
                    lps[0:NCLS, 0:BL], lhsT=wc2_s[:], rhs=hT[:], start=True, stop=True
                )
                nc.vector.tensor_tensor(
                    out=lgT[:],
                    in0=lps[0:NCLS, 0:BL],
                    in1=bc2_s[:].to_broadcast([NCLS, BL]),
                    op=ADD,
                )
            nc.sync.dma_start(out_d.ap(), lgT[:])

    nc.compile()
    return nc


def prepare_in_maps(input_ids, emb, Wq, bq, Wk, bk, Wv, bv, Wc1, bc1, Wc2, bc2):
    pe = _pos_encoding().astype(np.float64)
    pQ = (pe @ Wq.astype(np.float64) + bq.astype(np.float64)).astype(np.float32)
    pK = (pe @ Wk.astype(np.float64) + bk.astype(np.float64)).astype(np.float32)
    pV = (pe @ Wv.astype(np.float64) + bv.astype(np.float64)).astype(np.float32)

    emb16 = emb.astype(ml_dtypes.bfloat16)

    def chunk_w(w):  # [D, D] -> [128, 2, D] bf16 with [p,k,j] = w[k*128+p, j]
        return np.ascontiguousarray(
            w.reshape(2, 128, D).transpose(1, 0, 2).astype(ml_dtypes.bfloat16)
        )

    wq16 = chunk_w(Wq)
    wk16 = chunk_w(Wk)
    wv16 = chunk_w(Wv)

    def chunk_pT(p):  # [S, D] -> [128, 2, S] f32 with [p_,m,s] = p[s, m*128+p_]
        return np.ascontiguousarray(p.T.reshape(2, 128, S).transpose(1, 0, 2)).astype(
            np.float32
        )

    pqt = chunk_pT(pQ).astype(ml_dtypes.bfloat16)
    pkt = chunk_pT(pK).astype(ml_dtypes.bfloat16)
    # pv16[p, sc, d] = pV[sc*128+p, d]
    pv16 = np.ascontiguousarray(
        pV.reshape(SCH, 128, D).transpose(1, 0, 2)
    ).astype(ml_dtypes.bfloat16)
    # 1/S of the mean pooling is folded in here
    wc1 = np.ascontiguousarray(
        (Wc1 / np.float32(S)).reshape(2, 128, HID).transpose(1, 0, 2).astype(np.float32)
    )
    bc1c = np.ascontiguousarray(bc1.reshape(HID, 1).astype(np.float32))
    wc2 = np.ascontiguousarray(Wc2.astype(np.float32))
    bc2c = np.ascontiguousarray(bc2.reshape(NCLS, 1).astype(np.float32))

    in_maps = []
    for c in range(NCORES):
        m = dict(
            wq16=wq16,
            wk16=wk16,
            wv16=wv16,
            pqt=pqt,
            pkt=pkt,
            pv16=pv16,
            wc1=wc1,
            bc1c=bc1c,
            wc2=wc2,
            bc2c=bc2c,
        )
        for n in range(BL):
            e = emb16[input_ids[c * BL + n]]  # [S, D] bf16, host-side gather
            # et[p, k, s] = e[s, k*128+p]
            m[f"et{n}"] = np.ascontiguousarray(
                e.T.reshape(2, 128, S).transpose(1, 0, 2)
            )
        in_maps.append(m)
    return in_maps


_NC_CACHE = {}


def kernel(**inputs):
    inputs = {k: np.asarray(v) for k, v in inputs.items()}
    if "nc" not in _NC_CACHE:
        _NC_CACHE["nc"] = build_module()
    nc = _NC_CACHE["nc"]
    in_maps = prepare_in_maps(**inputs)
    res = run_bass_kernel_spmd(nc, in_maps, core_ids=list(range(NCORES)))
    out = np.empty((B, NCLS), dtype=np.float32)
    for c in range(NCORES):
        out[c * BL:(c + 1) * BL] = res.results[c]["lgt"].T
    return out


# revision 10
# speedup vs baseline: 1.2982x; 1.0006x over previous
"""Trainium2 Bass kernel for CustomAttentionClassifier.

Model (see reference): x = emb[ids] + pe; Q/K/V = x@W + b;
attn = softmax(QK^T/16); pooled = mean_s(attn @ V); logits = relu(pooled@Wc1+bc1)@Wc2+bc2.

Sharding: data-parallel over batch, B=64 -> 8 cores x 8 batches.

v2 restructuring (vs the gather-based v1):
- The embedding lookup + transpose happens on HOST: each core receives
  e^T per batch ([128, 2, S] bf16), so the device never touches the 15.6MB
  table and the pathological SWDGE transposed gather is gone.
- Host precomputes pQ = pe@Wq+bq (fp64) etc., so the device only adds the
  position-independent parts: Q^T = Wq^T e^T + pQ^T.
- mean-pool commutes with attn@V:  pooled = (mean_s attn) @ V, so the
  whole [S,S]x[S,D] context matmul is replaced by per-batch attention
  column means (abar). Scores are computed s-on-partitions; row sums come
  from a free-dim DVE reduction; abar^T = sum_s (1/rowsum_s) exp[s,:] is
  a PE matmul with a block-diagonal masked lhsT that accumulates all 8
  batches into one [8, 512] PSUM tile.
- The 1/S of the mean is folded into Wc1 on host; pe@Wv+bv is folded into
  the pooled matmul accumulation (extra lhsT terms), so V = e@Wv only.
"""

import numpy as np
import ml_dtypes

import concourse.bass as bass
import concourse.tile as tile
from concourse import bacc, mybir
from concourse.bass_utils import run_bass_kernel_spmd

V, D, S, B = 30522, 256, 512, 64
HID, NCLS = 128, 16
NCORES = 8
BL = B // NCORES          # 8 batches per core
SCH = S // 128            # 4 s/t chunks per batch

f32 = mybir.dt.float32
bf16 = mybir.dt.bfloat16

# knobs
import os as _os
STAGE = int(_os.environ.get("STAGE", "7"))  # debug truncation: 7=full


def _pos_encoding():
    pos = np.arange(S)[:, None].astype(np.float64)
    div = np.exp(np.arange(0, D, 2).astype(np.float64) * (-np.log(10000.0) / D))
    pe = np.zeros((S, D), dtype=np.float64)
    pe[:, 0::2] = np.sin(pos * div)
    pe[:, 1::2] = np.cos(pos * div)
    # match the reference, which builds pe in float32
    return pe.astype(np.float32)


def build_module():
    nc = bacc.Bacc("TRN2", target_bir_lowering=False, debug=False)

    hdr_d = nc.dram_tensor("hdr", [128, 2, S + 2 * D], bf16, kind="ExternalInput")
    et_d = [None] + [
        nc.dram_tensor(f"et{n}", [128, 2, S], bf16, kind="ExternalInput")
        for n in range(1, BL)
    ]
    eye_d = nc.dram_tensor("eye8", [BL, BL], bf16, kind="ExternalInput")
    wv_d = nc.dram_tensor("wv16", [128, 2, D], bf16, kind="ExternalInput")
    pqt_d = nc.dram_tensor("pqt", [128, 2, S], bf16, kind="ExternalInput")
    pkt_d = nc.dram_tensor("pkt", [128, 2, S], bf16, kind="ExternalInput")
    pv_d = nc.dram_tensor("pv16", [128, SCH, D], bf16, kind="ExternalInput")
    wc1_d = nc.dram_tensor("wc1", [128, 2, HID], f32, kind="ExternalInput")
    bc1_d = nc.dram_tensor("bc1c", [128, 1], f32, kind="ExternalInput")
    wc2_d = nc.dram_tensor("wc2", [128, NCLS], f32, kind="ExternalInput")
    bc2_d = nc.dram_tensor("bc2c", [16, 1], f32, kind="ExternalInput")
    out_d = nc.dram_tensor("lgt", [NCLS, BL], f32, kind="ExternalOutput")

    ADD = mybir.AluOpType.add
    EXP = mybir.ActivationFunctionType.Exp
    RELU = mybir.ActivationFunctionType.Relu
    AX = mybir.AxisListType.X

    with tile.TileContext(nc) as tc:
        with (
            tc.tile_pool(name="const", bufs=1) as cp,
            tc.tile_pool(name="work", bufs=3) as wp,
            tc.tile_pool(name="psA", bufs=2, space="PSUM") as psA,
            tc.tile_pool(name="psB", bufs=3, space="PSUM") as psB,
            tc.tile_pool(name="psC", bufs=1, space="PSUM") as psC,
        ):
            hdr_s = cp.tile([128, 2, S + 2 * D], bf16, tag="hdr")
            eye_s = cp.tile([BL, BL], bf16, tag="eye8")
            wv_s = cp.tile([128, 2, D], bf16, tag="wv")
            pqt_s = cp.tile([128, 2, S], bf16, tag="pqt")
            pkt_s = cp.tile([128, 2, S], bf16, tag="pkt")
            pv_s = cp.tile([128, SCH, D], bf16, tag="pv")
            wc1_s = cp.tile([128, 2, HID], f32, tag="wc1")
            bc1_s = cp.tile([128, 1], f32, tag="bc1")
            wc2_s = cp.tile([128, NCLS], f32, tag="wc2")
            bc2_s = cp.tile([16, 1], f32, tag="bc2")

            eTs = [hdr_s[:, :, 0:S]] + [
                cp.tile([128, 2, S], bf16, tag=f"eT{n}", name=f"eT{n}")
                for n in range(1, BL)
            ]
            qT = cp.tile([128, 2, BL * S], bf16, tag="qT")
            kT = cp.tile([128, 2, BL * S], bf16, tag="kT")
            vS = cp.tile([128, BL * SCH, D], bf16, tag="vS")
            rsum = cp.tile([128, BL, SCH], f32, tag="rsum")
            rc32 = cp.tile([128, BL, SCH], f32, tag="rc32")
            # block-diagonal masked lhsT for the abar matmuls:
            # rrbM[p, sc, n, col] = (col==n) * 1/rowsum_n[sc*128+p]
            rrbM = cp.tile([128, SCH, BL, BL], bf16, tag="rrbM")
            attnRows = cp.tile([BL, S], bf16, tag="attnRows")
            attnT = cp.tile([128, SCH, BL], bf16, tag="attnT")
            pooledT = cp.tile([128, 2, BL], f32, tag="pooledT")
            hT = cp.tile([128, BL], f32, tag="hT")
            lgT = cp.tile([16, BL], f32, tag="lgT")

            nc.sync.dma_start(hdr_s[:], hdr_d.ap())
            nc.sync.dma_start(pqt_s[:], pqt_d.ap())
            nc.sync.dma_start(pkt_s[:], pkt_d.ap())
            nc.sync.dma_start(wv_s[:], wv_d.ap())
            nc.sync.dma_start(eTs[1][:], et_d[1].ap())
            nc.sync.dma_start(eTs[2][:], et_d[2].ap())
            nc.sync.dma_start(pv_s[:], pv_d.ap())
            nc.sync.dma_start(wc1_s[:], wc1_d.ap())
            nc.sync.dma_start(bc1_s[:], bc1_d.ap())
            nc.sync.dma_start(wc2_s[:], wc2_d.ap())
            nc.sync.dma_start(bc2_s[:], bc2_d.ap())
            nc.sync.dma_start(eye_s[:], eye_d.ap())
            for n in range(3, BL):
                nc.sync.dma_start(eTs[n][:], et_d[n].ap())

            nc.gpsimd.memset(rrbM[:], 0.0)
            if STAGE < 7:
                nc.vector.memset(lgT[:], 0.0)

            psAB = psC.tile([128, S], f32, tag="AB")  # rows 0:BL used
            expTiles = [None] * BL

            def emit_abar(n):
                # abar row n (x S): sum_s exp[s, t] / rowsum[s], accumulated
                # into the shared psAB via the masked lhsT. One long
                # accumulation group across all batches.
                for sc in range(SCH):
                    nc.tensor.matmul(
                        psAB[0:BL, :],
                        lhsT=rrbM[:, sc, n, :],
                        rhs=expTiles[n][:, sc, :],
                        start=(n == 0 and sc == 0),
                        stop=(n == BL - 1 and sc == SCH - 1),
                        skip_group_check=True,
                    )

            for n in range(BL if STAGE >= 1 else 0):
                # ---- Q^T, K^T for batch n ----
                for wof, pT_s, oT in ((S, pqt_s, qT), (S + D, pkt_s, kT)):
                    for m in range(2):
                        ps = psB.tile([128, S], f32, tag="B")
                        for k in range(2):
                            nc.tensor.matmul(
                                ps[:],
                                lhsT=hdr_s[:, k, wof + m * 128:wof + (m + 1) * 128],
                                rhs=eTs[n][:, k, 0:S],
                                start=(k == 0),
                                stop=(k == 1),
                            )
                        nc.vector.tensor_tensor(
                            out=oT[:, m, n * S:(n + 1) * S],
                            in0=ps[:],
                            in1=pT_s[:, m, :],
                            op=ADD,
                        )

                # ---- V = e @ Wv for batch n (pe/bv part folded into pooled) ----
                if STAGE >= 2:
                    for sc in range(SCH):
                        psv = psB.tile([128, D], f32, tag="B")
                        for k in range(2):
                            nc.tensor.matmul(
                                psv[:],
                                lhsT=eTs[n][:, k, sc * 128:(sc + 1) * 128],
                                rhs=wv_s[:, k, :],
                                start=(k == 0),
                                stop=(k == 1),
                            )
                        nc.vector.tensor_copy(out=vS[:, n * SCH + sc, :], in_=psv[:])

                # ---- scores (s on partitions) + exp + rowsums ----
                if STAGE >= 3:
                    expT = wp.tile([128, SCH, S], bf16, tag="expT")
                    expTiles[n] = expT
                    for w in range(2):
                        ps = psA.tile([128, 2, S], f32, tag="A")
                        for i in range(2):
                            sc = 2 * w + i
                            for m in range(2):
                                nc.tensor.matmul(
                                    ps[:, i, :],
                                    lhsT=qT[:, m, n * S + sc * 128: n * S + (sc + 1) * 128],
                                    rhs=kT[:, m, n * S:(n + 1) * S],
                                    start=(m == 0),
                                    stop=(m == 1),
                                )
                        for i in range(2):
                            sc = 2 * w + i
                            nc.scalar.activation(
                                out=expT[:, sc, :],
                                in_=ps[:, i, :],
                                func=EXP,
                                scale=1.0 / 16.0,
                                accum_out=rsum[:, n, sc:sc + 1],
                            )
                    if n < BL - 1:
                        nc.vector.reciprocal(out=rc32[:, n, :], in_=rsum[:, n, :])
                        nc.vector.tensor_copy(out=rrbM[:, :, n, n], in_=rc32[:, n, :])
                    else:
                        # last batch: per-sc, so abar(7, sc) can start as soon
                        # as exp(7, sc) lands instead of after the whole chain
                        for sc in range(SCH):
                            nc.vector.reciprocal(
                                out=rc32[:, n, sc:sc + 1], in_=rsum[:, n, sc:sc + 1]
                            )
                            nc.vector.tensor_copy(
                                out=rrbM[:, sc, n, n:n + 1], in_=rc32[:, n, sc:sc + 1]
                            )

                    # software-pipelined by one batch: emit abar(n-1) here so
                    # PE never stalls waiting on this batch's exp/rowsum.
                    if STAGE >= 4 and n > 0:
                        emit_abar(n - 1)

            if STAGE >= 4:
                emit_abar(BL - 1)
                nc.scalar.copy(out=attnRows[0:BL, :], in_=psAB[0:BL, :])

            if STAGE >= 5:
                # transpose abar rows -> columns on PE: attnT[p, sc, b]
                for sc in range(SCH):
                    pst = psB.tile([128, BL], bf16, tag="B")
                    nc.tensor.transpose(
                        pst[:],
                        in_=attnRows[0:BL, sc * 128:(sc + 1) * 128],
                        identity=eye_s[:],
                    )
                    nc.scalar.copy(out=attnT[:, sc, :], in_=pst[:])

            if STAGE >= 6:
                # pooled^T (x S, folded into wc1): for each d-chunk, accumulate
                # all 8 batches into one [128, BL] psum (per-column groups).
                for dch in range(2):
                    psp = psB.tile([128, BL], f32, tag="B")
                    for n in range(BL):
                        for tc in range(SCH):
                            nc.tensor.matmul(
                                psp[:, n:n + 1],
                                lhsT=vS[:, n * SCH + tc, dch * 128:(dch + 1) * 128],
                                rhs=attnT[:, tc, n:n + 1],
                                start=(tc == 0),
                                stop=False,
                                skip_group_check=True,
                            )
                            nc.tensor.matmul(
                                psp[:, n:n + 1],
                                lhsT=pv_s[:, tc, dch * 128:(dch + 1) * 128],
                                rhs=attnT[:, tc, n:n + 1],
                                start=False,
                                stop=(tc == SCH - 1),
                                skip_group_check=True,
                            )
                    nc.scalar.copy(out=pooledT[:, dch, :], in_=psp[:, 0:BL])

            # ---- classifier ----
            if STAGE >= 7:
                hps = psB.tile([128, BL], f32, tag="B")
                for k in range(2):
                    nc.tensor.matmul(
                        hps[:, 0:BL],
                        lhsT=wc1_s[:, k, :],
                        rhs=pooledT[:, k, :],
                        start=(k == 0),
                        stop=(k == 1),
                    )
                nc.scalar.activation(
                    out=hT[:], in_=hps[:, 0:BL], func=RELU, bias=bc1_s[:]
                )

                lps = psB.tile([128, BL], f32, tag="B")
                nc.tensor.matmul(
                    lps[0:NCLS, 0:BL], lhsT=wc2_s[:], rhs=hT[:], start=True, stop=True
                )
                nc.scalar.activation(
                    out=lgT[:],
                    in_=lps[0:NCLS, 0:BL],
                    func=mybir.ActivationFunctionType.Identity,
                    bias=bc2_s[:],
                )
            nc.sync.dma_start(out_d.ap(), lgT[:])

    nc.compile()
    return nc


def prepare_in_maps(input_ids, emb, Wq, bq, Wk, bk, Wv, bv, Wc1, bc1, Wc2, bc2):
    pe = _pos_encoding().astype(np.float64)
    pQ = (pe @ Wq.astype(np.float64) + bq.astype(np.float64)).astype(np.float32)
    pK = (pe @ Wk.astype(np.float64) + bk.astype(np.float64)).astype(np.float32)
    pV = (pe @ Wv.astype(np.float64) + bv.astype(np.float64)).astype(np.float32)

    emb16 = emb.astype(ml_dtypes.bfloat16)

    def chunk_w(w):  # [D, D] -> [128, 2, D] bf16 with [p,k,j] = w[k*128+p, j]
        return np.ascontiguousarray(
            w.reshape(2, 128, D).transpose(1, 0, 2).astype(ml_dtypes.bfloat16)
        )

    wq16 = chunk_w(Wq)
    wk16 = chunk_w(Wk)
    wv16 = chunk_w(Wv)
    eye8 = np.eye(BL, dtype=ml_dtypes.bfloat16)

    def chunk_pT(p):  # [S, D] -> [128, 2, S] f32 with [p_,m,s] = p[s, m*128+p_]
        return np.ascontiguousarray(p.T.reshape(2, 128, S).transpose(1, 0, 2)).astype(
            np.float32
        )

    pqt = chunk_pT(pQ).astype(ml_dtypes.bfloat16)
    pkt = chunk_pT(pK).astype(ml_dtypes.bfloat16)
    # pv16[p, sc, d] = pV[sc*128+p, d]
    pv16 = np.ascontiguousarray(
        pV.reshape(SCH, 128, D).transpose(1, 0, 2)
    ).astype(ml_dtypes.bfloat16)
    # 1/S of the mean pooling is folded in here
    wc1 = np.ascontiguousarray(
        (Wc1 / np.float32(S)).reshape(2, 128, HID).transpose(1, 0, 2).astype(np.float32)
    )
    bc1c = np.ascontiguousarray(bc1.reshape(HID, 1).astype(np.float32))
    wc2 = np.ascontiguousarray(Wc2.astype(np.float32))
    bc2c = np.ascontiguousarray(bc2.reshape(NCLS, 1).astype(np.float32))

    in_maps = []
    for c in range(NCORES):
        m = dict(
            eye8=eye8,
            wv16=wv16,
            pqt=pqt,
            pkt=pkt,
            pv16=pv16,
            wc1=wc1,
            bc1c=bc1c,
            wc2=wc2,
            bc2c=bc2c,
        )
        for n in range(BL):
            e = emb16[input_ids[c * BL + n]]  # [S, D] bf16, host-side gather
            # et[p, k, s] = e[s, k*128+p]
            et = np.ascontiguousarray(e.T.reshape(2, 128, S).transpose(1, 0, 2))
            if n == 0:
                m["hdr"] = np.ascontiguousarray(
                    np.concatenate([et, wq16, wk16], axis=2)
                )
            else:
                m[f"et{n}"] = et
        in_maps.append(m)
    return in_maps


_NC_CACHE = {}


def kernel(**inputs):
    inputs = {k: np.asarray(v) for k, v in inputs.items()}
    if "nc" not in _NC_CACHE:
        _NC_CACHE["nc"] = build_module()
    nc = _NC_CACHE["nc"]
    in_maps = prepare_in_maps(**inputs)
    res = run_bass_kernel_spmd(nc, in_maps, core_ids=list(range(NCORES)))
    out = np.empty((B, NCLS), dtype=np.float32)
    for c in range(NCORES):
        out[c * BL:(c + 1) * BL] = res.results[c]["lgt"].T
    return out


# revision 12
# speedup vs baseline: 1.3569x; 1.0453x over previous
"""Trainium2 Bass kernel for CustomAttentionClassifier.

Model (see reference): x = emb[ids] + pe; Q/K/V = x@W + b;
attn = softmax(QK^T/16); pooled = mean_s(attn @ V); logits = relu(pooled@Wc1+bc1)@Wc2+bc2.

Sharding: data-parallel over batch, B=64 -> 8 cores x 8 batches.

v2 restructuring (vs the gather-based v1):
- The embedding lookup + transpose happens on HOST: each core receives
  e^T per batch ([128, 2, S] bf16), so the device never touches the 15.6MB
  table and the pathological SWDGE transposed gather is gone.
- Host precomputes pQ = pe@Wq+bq (fp64) etc., so the device only adds the
  position-independent parts: Q^T = Wq^T e^T + pQ^T.
- mean-pool commutes with attn@V:  pooled = (mean_s attn) @ V, so the
  whole [S,S]x[S,D] context matmul is replaced by per-batch attention
  column means (abar). Scores are computed s-on-partitions; row sums come
  from a free-dim DVE reduction; abar^T = sum_s (1/rowsum_s) exp[s,:] is
  a PE matmul with a block-diagonal masked lhsT that accumulates all 8
  batches into one [8, 512] PSUM tile.
- The 1/S of the mean is folded into Wc1 on host; pe@Wv+bv is folded into
  the pooled matmul accumulation (extra lhsT terms), so V = e@Wv only.
"""

import numpy as np
import ml_dtypes

import concourse.bass as bass
import concourse.tile as tile
from concourse import bacc, mybir
from concourse.bass_utils import run_bass_kernel_spmd

V, D, S, B = 30522, 256, 512, 64
HID, NCLS = 128, 16
NCORES = 8
BL = B // NCORES          # 8 batches per core
SCH = S // 128            # 4 s/t chunks per batch

f32 = mybir.dt.float32
bf16 = mybir.dt.bfloat16

# knobs
import os as _os
STAGE = int(_os.environ.get("STAGE", "7"))  # debug truncation: 7=full


def _pos_encoding():
    pos = np.arange(S)[:, None].astype(np.float64)
    div = np.exp(np.arange(0, D, 2).astype(np.float64) * (-np.log(10000.0) / D))
    pe = np.zeros((S, D), dtype=np.float64)
    pe[:, 0::2] = np.sin(pos * div)
    pe[:, 1::2] = np.cos(pos * div)
    # match the reference, which builds pe in float32
    return pe.astype(np.float32)


def build_module():
    nc = bacc.Bacc("TRN2", target_bir_lowering=False, debug=False)

    wq_d = nc.dram_tensor("wq16", [128, 2, D], bf16, kind="ExternalInput")
    wk_d = nc.dram_tensor("wk16", [128, 2, D], bf16, kind="ExternalInput")
    et_d = [
        nc.dram_tensor(f"et{n}", [128, 2, S], bf16, kind="ExternalInput")
        for n in range(BL)
    ]
    eye_d = nc.dram_tensor("eye8", [BL, BL], bf16, kind="ExternalInput")
    wv_d = nc.dram_tensor("wv16", [128, 2, D], bf16, kind="ExternalInput")
    pqt_d = nc.dram_tensor("pqt", [128, 2, S], bf16, kind="ExternalInput")
    pkt_d = nc.dram_tensor("pkt", [128, 2, S], bf16, kind="ExternalInput")
    pv_d = nc.dram_tensor("pv16", [128, SCH, D], bf16, kind="ExternalInput")
    wc1_d = nc.dram_tensor("wc1", [128, 2, HID], f32, kind="ExternalInput")
    bc1_d = nc.dram_tensor("bc1c", [128, 1], f32, kind="ExternalInput")
    wc2_d = nc.dram_tensor("wc2", [128, NCLS], f32, kind="ExternalInput")
    bc2_d = nc.dram_tensor("bc2c", [16, 1], f32, kind="ExternalInput")
    out_d = nc.dram_tensor("lgt", [NCLS, BL], f32, kind="ExternalOutput")

    ADD = mybir.AluOpType.add
    EXP = mybir.ActivationFunctionType.Exp
    RELU = mybir.ActivationFunctionType.Relu
    AX = mybir.AxisListType.X

    with tile.TileContext(nc) as tc:
        with (
            tc.tile_pool(name="const", bufs=1) as cp,
            tc.tile_pool(name="work", bufs=3) as wp,
            tc.tile_pool(name="psA", bufs=2, space="PSUM") as psA,
            tc.tile_pool(name="psB", bufs=3, space="PSUM") as psB,
            tc.tile_pool(name="psC", bufs=1, space="PSUM") as psC,
        ):
            wq_s = cp.tile([128, 2, D], bf16, tag="wq")
            wk_s = cp.tile([128, 2, D], bf16, tag="wk")
            eye_s = cp.tile([BL, BL], bf16, tag="eye8")
            wv_s = cp.tile([128, 2, D], bf16, tag="wv")
            pqt_s = cp.tile([128, 2, S], bf16, tag="pqt")
            pkt_s = cp.tile([128, 2, S], bf16, tag="pkt")
            pv_s = cp.tile([128, SCH, D], bf16, tag="pv")
            wc1_s = cp.tile([128, 2, HID], f32, tag="wc1")
            bc1_s = cp.tile([128, 1], f32, tag="bc1")
            wc2_s = cp.tile([128, NCLS], f32, tag="wc2")
            bc2_s = cp.tile([16, 1], f32, tag="bc2")

            eTs = [
                cp.tile([128, 2, S], bf16, tag=f"eT{n}", name=f"eT{n}")
                for n in range(BL)
            ]
            qT = cp.tile([128, 2, BL * S], bf16, tag="qT")
            kT = cp.tile([128, 2, BL * S], bf16, tag="kT")
            vS = cp.tile([128, BL * SCH, D], bf16, tag="vS")
            rsum = cp.tile([128, BL, SCH], f32, tag="rsum")
            rc32 = cp.tile([128, BL, SCH], f32, tag="rc32")
            # block-diagonal masked lhsT for the abar matmuls:
            # rrbM[p, sc, n, col] = (col==n) * 1/rowsum_n[sc*128+p]
            rrbM = cp.tile([128, SCH, BL, BL], bf16, tag="rrbM")
            attnRows = cp.tile([BL, S], bf16, tag="attnRows")
            attnT = cp.tile([128, SCH, BL], bf16, tag="attnT")
            pooledT = cp.tile([128, 2, BL], f32, tag="pooledT")
            hT = cp.tile([128, BL], f32, tag="hT")
            lgT = cp.tile([16, BL], f32, tag="lgT")

            nc.sync.dma_start(wq_s[:], wq_d.ap())
            nc.sync.dma_start(eTs[0][:], et_d[0].ap())
            nc.sync.dma_start(wk_s[:], wk_d.ap())
            nc.sync.dma_start(pqt_s[:], pqt_d.ap())
            nc.sync.dma_start(pkt_s[:], pkt_d.ap())
            nc.sync.dma_start(wv_s[:], wv_d.ap())
            nc.sync.dma_start(eTs[1][:], et_d[1].ap())
            nc.sync.dma_start(eTs[2][:], et_d[2].ap())
            nc.sync.dma_start(pv_s[:], pv_d.ap())
            nc.sync.dma_start(wc1_s[:], wc1_d.ap())
            nc.sync.dma_start(bc1_s[:], bc1_d.ap())
            nc.sync.dma_start(wc2_s[:], wc2_d.ap())
            nc.sync.dma_start(bc2_s[:], bc2_d.ap())
            nc.sync.dma_start(eye_s[:], eye_d.ap())
            for n in range(3, BL):
                nc.sync.dma_start(eTs[n][:], et_d[n].ap())

            nc.gpsimd.memset(rrbM[:], 0.0)
            if STAGE < 7:
                nc.vector.memset(lgT[:], 0.0)

            # warm the PE p-state during the initial DMA wait: six dummy
            # matmuls on the (memset) rrbM tile keep the engine busy so real
            # matmuls start at full clock. Results are never read.
            warm = psB.tile([128, S], f32, tag="B")
            for _ in range(6):
                nc.tensor.matmul(
                    warm[0:BL, 0:BL * BL],
                    lhsT=rrbM[:, 0, 0, :],
                    rhs=rrbM[:, 0, :, :],
                    start=True,
                    stop=True,
                )

            psAB = psC.tile([128, S], f32, tag="AB")  # rows 0:BL used
            expTiles = [None] * BL

            def emit_abar(n):
                # abar row n (x S): sum_s exp[s, t] / rowsum[s], accumulated
                # into the shared psAB via the masked lhsT. One long
                # accumulation group across all batches.
                for sc in range(SCH):
                    nc.tensor.matmul(
                        psAB[0:BL, :],
                        lhsT=rrbM[:, sc, n, :],
                        rhs=expTiles[n][:, sc, :],
                        start=(n == 0 and sc == 0),
                        stop=(n == BL - 1 and sc == SCH - 1),
                        skip_group_check=True,
                    )

            for n in range(BL if STAGE >= 1 else 0):
                # ---- Q^T, K^T for batch n ----
                for w_s, pT_s, oT in ((wq_s, pqt_s, qT), (wk_s, pkt_s, kT)):
                    for m in range(2):
                        ps = psB.tile([128, S], f32, tag="B")
                        for k in range(2):
                            nc.tensor.matmul(
                                ps[:],
                                lhsT=w_s[:, k, m * 128:(m + 1) * 128],
                                rhs=eTs[n][:, k, 0:S],
                                start=(k == 0),
                                stop=(k == 1),
                            )
                        nc.vector.tensor_tensor(
                            out=oT[:, m, n * S:(n + 1) * S],
                            in0=ps[:],
                            in1=pT_s[:, m, :],
                            op=ADD,
                        )

                # ---- V = e @ Wv for batch n (pe/bv part folded into pooled) ----
                if STAGE >= 2:
                    for sc in range(SCH):
                        psv = psB.tile([128, D], f32, tag="B")
                        for k in range(2):
                            nc.tensor.matmul(
                                psv[:],
                                lhsT=eTs[n][:, k, sc * 128:(sc + 1) * 128],
                                rhs=wv_s[:, k, :],
                                start=(k == 0),
                                stop=(k == 1),
                            )
                        nc.vector.tensor_copy(out=vS[:, n * SCH + sc, :], in_=psv[:])

                # ---- scores (s on partitions) + exp + rowsums ----
                if STAGE >= 3:
                    expT = wp.tile([128, SCH, S], bf16, tag="expT")
                    expTiles[n] = expT
                    for w in range(2):
                        ps = psA.tile([128, 2, S], f32, tag="A")
                        for i in range(2):
                            sc = 2 * w + i
                            for m in range(2):
                                nc.tensor.matmul(
                                    ps[:, i, :],
                                    lhsT=qT[:, m, n * S + sc * 128: n * S + (sc + 1) * 128],
                                    rhs=kT[:, m, n * S:(n + 1) * S],
                                    start=(m == 0),
                                    stop=(m == 1),
                                )
                        for i in range(2):
                            sc = 2 * w + i
                            nc.scalar.activation(
                                out=expT[:, sc, :],
                                in_=ps[:, i, :],
                                func=EXP,
                                scale=1.0 / 16.0,
                                accum_out=rsum[:, n, sc:sc + 1],
                            )
                    if n < BL - 1:
                        nc.vector.reciprocal(out=rc32[:, n, :], in_=rsum[:, n, :])
                        nc.vector.tensor_copy(out=rrbM[:, :, n, n], in_=rc32[:, n, :])
                    else:
                        # last batch: per-sc, so abar(7, sc) can start as soon
                        # as exp(7, sc) lands instead of after the whole chain
                        for sc in range(SCH):
                            nc.vector.reciprocal(
                                out=rc32[:, n, sc:sc + 1], in_=rsum[:, n, sc:sc + 1]
                            )
                            nc.vector.tensor_copy(
                                out=rrbM[:, sc, n, n:n + 1], in_=rc32[:, n, sc:sc + 1]
                            )

                    # software-pipelined by one batch: emit abar(n-1) here so
                    # PE never stalls waiting on this batch's exp/rowsum.
                    if STAGE >= 4 and n > 0:
                        emit_abar(n - 1)

            if STAGE >= 4:
                emit_abar(BL - 1)
                nc.scalar.copy(out=attnRows[0:BL, :], in_=psAB[0:BL, :])

            if STAGE >= 5:
                # transpose abar rows -> columns on PE: attnT[p, sc, b]
                for sc in range(SCH):
                    pst = psB.tile([128, BL], bf16, tag="B")
                    nc.tensor.transpose(
                        pst[:],
                        in_=attnRows[0:BL, sc * 128:(sc + 1) * 128],
                        identity=eye_s[:],
                    )
                    nc.scalar.copy(out=attnT[:, sc, :], in_=pst[:])

            if STAGE >= 6:
                # pooled^T (x S, folded into wc1): for each d-chunk, accumulate
                # all 8 batches into one [128, BL] psum (per-column groups).
                psp = psB.tile([128, 2, BL], f32, tag="B")
                for dch in range(2):
                    for n in range(BL):
                        for tc in range(SCH):
                            nc.tensor.matmul(
                                psp[:, dch, n:n + 1],
                                lhsT=vS[:, n * SCH + tc, dch * 128:(dch + 1) * 128],
                                rhs=attnT[:, tc, n:n + 1],
                                start=(tc == 0),
                                stop=False,
                                skip_group_check=True,
                            )
                            nc.tensor.matmul(
                                psp[:, dch, n:n + 1],
                                lhsT=pv_s[:, tc, dch * 128:(dch + 1) * 128],
                                rhs=attnT[:, tc, n:n + 1],
                                start=False,
                                stop=(tc == SCH - 1),
                                skip_group_check=True,
                            )
                nc.scalar.copy(out=pooledT[:], in_=psp[:])

            # ---- classifier ----
            if STAGE >= 7:
                hps = psB.tile([128, BL], f32, tag="B")
                for k in range(2):
                    nc.tensor.matmul(
                        hps[:, 0:BL],
                        lhsT=wc1_s[:, k, :],
                        rhs=pooledT[:, k, :],
                        start=(k == 0),
                        stop=(k == 1),
                    )
                nc.scalar.activation(
                    out=hT[:], in_=hps[:, 0:BL], func=RELU, bias=bc1_s[:]
                )

                lps = psB.tile([128, BL], f32, tag="B")
                nc.tensor.matmul(
                    lps[0:NCLS, 0:BL], lhsT=wc2_s[:], rhs=hT[:], start=True, stop=True
                )
                nc.scalar.activation(
                    out=lgT[:],
                    in_=lps[0:NCLS, 0:BL],
                    func=mybir.ActivationFunctionType.Identity,
                    bias=bc2_s[:],
                )
            nc.sync.dma_start(out_d.ap(), lgT[:])

    nc.compile()
    return nc


def prepare_in_maps(input_ids, emb, Wq, bq, Wk, bk, Wv, bv, Wc1, bc1, Wc2, bc2):
    pe = _pos_encoding().astype(np.float64)
    pQ = (pe @ Wq.astype(np.float64) + bq.astype(np.float64)).astype(np.float32)
    pK = (pe @ Wk.astype(np.float64) + bk.astype(np.float64)).astype(np.float32)
    pV = (pe @ Wv.astype(np.float64) + bv.astype(np.float64)).astype(np.float32)

    emb16 = emb.astype(ml_dtypes.bfloat16)

    def chunk_w(w):  # [D, D] -> [128, 2, D] bf16 with [p,k,j] = w[k*128+p, j]
        return np.ascontiguousarray(
            w.reshape(2, 128, D).transpose(1, 0, 2).astype(ml_dtypes.bfloat16)
        )

    wq16 = chunk_w(Wq)
    wk16 = chunk_w(Wk)
    wv16 = chunk_w(Wv)
    eye8 = np.eye(BL, dtype=ml_dtypes.bfloat16)

    def chunk_pT(p):  # [S, D] -> [128, 2, S] f32 with [p_,m,s] = p[s, m*128+p_]
        return np.ascontiguousarray(p.T.reshape(2, 128, S).transpose(1, 0, 2)).astype(
            np.float32
        )

    pqt = chunk_pT(pQ).astype(ml_dtypes.bfloat16)
    pkt = chunk_pT(pK).astype(ml_dtypes.bfloat16)
    # pv16[p, sc, d] = pV[sc*128+p, d]
    pv16 = np.ascontiguousarray(
        pV.reshape(SCH, 128, D).transpose(1, 0, 2)
    ).astype(ml_dtypes.bfloat16)
    # 1/S of the mean pooling is folded in here
    wc1 = np.ascontiguousarray(
        (Wc1 / np.float32(S)).reshape(2, 128, HID).transpose(1, 0, 2).astype(np.float32)
    )
    bc1c = np.ascontiguousarray(bc1.reshape(HID, 1).astype(np.float32))
    wc2 = np.ascontiguousarray(Wc2.astype(np.float32))
    bc2c = np.ascontiguousarray(bc2.reshape(NCLS, 1).astype(np.float32))

    in_maps = []
    for c in range(NCORES):
        m = dict(
            eye8=eye8,
            wq16=wq16,
            wk16=wk16,
            wv16=wv16,
            pqt=pqt,
            pkt=pkt,
            pv16=pv16,
            wc1=wc1,
            bc1c=bc1c,
            wc2=wc2,
            bc2c=bc2c,
        )
        for n in range(BL):
            e = emb16[input_ids[c * BL + n]]  # [S, D] bf16, host-side gather
            # et[p, k, s] = e[s, k*128+p]
            m[f"et{n}"] = np.ascontiguousarray(
                e.T.reshape(2, 128, S).transpose(1, 0, 2)
            )
        in_maps.append(m)
    return in_maps


_NC_CACHE = {}


def kernel(**inputs):
    inputs = {k: np.asarray(v) for k, v in inputs.items()}
    if "nc" not in _NC_CACHE:
        _NC_CACHE["nc"] = build_module()
    nc = _NC_CACHE["nc"]
    in_maps = prepare_in_maps(**inputs)
    res = run_bass_kernel_spmd(nc, in_maps, core_ids=list(range(NCORES)))
    out = np.empty((B, NCLS), dtype=np.float32)
    for c in range(NCORES):
        out[c * BL:(c + 1) * BL] = res.results[c]["lgt"].T
    return out


# revision 13
# speedup vs baseline: 1.3707x; 1.0101x over previous
"""Trainium2 Bass kernel for CustomAttentionClassifier.

Model (see reference): x = emb[ids] + pe; Q/K/V = x@W + b;
attn = softmax(QK^T/16); pooled = mean_s(attn @ V); logits = relu(pooled@Wc1+bc1)@Wc2+bc2.

Sharding: data-parallel over batch, B=64 -> 8 cores x 8 batches.

v2 restructuring (vs the gather-based v1):
- The embedding lookup + transpose happens on HOST: each core receives
  e^T per batch ([128, 2, S] bf16), so the device never touches the 15.6MB
  table and the pathological SWDGE transposed gather is gone.
- Host precomputes pQ = pe@Wq+bq (fp64) etc., so the device only adds the
  position-independent parts: Q^T = Wq^T e^T + pQ^T.
- mean-pool commutes with attn@V:  pooled = (mean_s attn) @ V, so the
  whole [S,S]x[S,D] context matmul is replaced by per-batch attention
  column means (abar). Scores are computed s-on-partitions; row sums come
  from a free-dim DVE reduction; abar^T = sum_s (1/rowsum_s) exp[s,:] is
  a PE matmul with a block-diagonal masked lhsT that accumulates all 8
  batches into one [8, 512] PSUM tile.
- The 1/S of the mean is folded into Wc1 on host; pe@Wv+bv is folded into
  the pooled matmul accumulation (extra lhsT terms), so V = e@Wv only.
"""

import numpy as np
import ml_dtypes

import concourse.bass as bass
import concourse.tile as tile
from concourse import bacc, mybir
from concourse.bass_utils import run_bass_kernel_spmd

V, D, S, B = 30522, 256, 512, 64
HID, NCLS = 128, 16
NCORES = 8
BL = B // NCORES          # 8 batches per core
SCH = S // 128            # 4 s/t chunks per batch

f32 = mybir.dt.float32
bf16 = mybir.dt.bfloat16

# knobs
import os as _os
STAGE = int(_os.environ.get("STAGE", "7"))  # debug truncation: 7=full


def _pos_encoding():
    pos = np.arange(S)[:, None].astype(np.float64)
    div = np.exp(np.arange(0, D, 2).astype(np.float64) * (-np.log(10000.0) / D))
    pe = np.zeros((S, D), dtype=np.float64)
    pe[:, 0::2] = np.sin(pos * div)
    pe[:, 1::2] = np.cos(pos * div)
    # match the reference, which builds pe in float32
    return pe.astype(np.float32)


def build_module():
    nc = bacc.Bacc("TRN2", target_bir_lowering=False, debug=False)

    wq_d = nc.dram_tensor("wq16", [128, 2, D], bf16, kind="ExternalInput")
    wk_d = nc.dram_tensor("wk16", [128, 2, D], bf16, kind="ExternalInput")
    et_d = [
        nc.dram_tensor(f"et{n}", [128, 2, S], bf16, kind="ExternalInput")
        for n in range(BL)
    ]
    eye_d = nc.dram_tensor("eye8", [BL, BL], bf16, kind="ExternalInput")
    wv_d = nc.dram_tensor("wv16", [128, 2, D], bf16, kind="ExternalInput")
    pqt_d = nc.dram_tensor("pqt", [128, 2, S], bf16, kind="ExternalInput")
    pkt_d = nc.dram_tensor("pkt", [128, 2, S], bf16, kind="ExternalInput")
    pv_d = nc.dram_tensor("pv16", [128, SCH, D], bf16, kind="ExternalInput")
    wc1_d = nc.dram_tensor("wc1", [128, 2, HID], f32, kind="ExternalInput")
    bc1_d = nc.dram_tensor("bc1c", [128, 1], f32, kind="ExternalInput")
    wc2_d = nc.dram_tensor("wc2", [128, NCLS], f32, kind="ExternalInput")
    bc2_d = nc.dram_tensor("bc2c", [16, 1], f32, kind="ExternalInput")
    out_d = nc.dram_tensor("lgt", [NCLS, BL], f32, kind="ExternalOutput")

    ADD = mybir.AluOpType.add
    EXP = mybir.ActivationFunctionType.Exp
    RELU = mybir.ActivationFunctionType.Relu
    AX = mybir.AxisListType.X

    with tile.TileContext(nc) as tc:
        with (
            tc.tile_pool(name="const", bufs=1) as cp,
            tc.tile_pool(name="work", bufs=3) as wp,
            tc.tile_pool(name="psA", bufs=2, space="PSUM") as psA,
            tc.tile_pool(name="psB", bufs=3, space="PSUM") as psB,
            tc.tile_pool(name="psC", bufs=1, space="PSUM") as psC,
        ):
            wq_s = cp.tile([128, 2, D], bf16, tag="wq")
            wk_s = cp.tile([128, 2, D], bf16, tag="wk")
            eye_s = cp.tile([BL, BL], bf16, tag="eye8")
            wv_s = cp.tile([128, 2, D], bf16, tag="wv")
            pqt_s = cp.tile([128, 2, S], bf16, tag="pqt")
            pkt_s = cp.tile([128, 2, S], bf16, tag="pkt")
            pv_s = cp.tile([128, SCH, D], bf16, tag="pv")
            wc1_s = cp.tile([128, 2, HID], f32, tag="wc1")
            bc1_s = cp.tile([128, 1], f32, tag="bc1")
            wc2_s = cp.tile([128, NCLS], f32, tag="wc2")
            bc2_s = cp.tile([16, 1], f32, tag="bc2")

            eTs = [
                cp.tile([128, 2, S], bf16, tag=f"eT{n}", name=f"eT{n}")
                for n in range(BL)
            ]
            qT = cp.tile([128, 2, BL * S], bf16, tag="qT")
            kT = cp.tile([128, 2, BL * S], bf16, tag="kT")
            vS = cp.tile([128, BL * SCH, D], bf16, tag="vS")
            rsum = cp.tile([128, BL, SCH], f32, tag="rsum")
            rc32 = cp.tile([128, BL, SCH], f32, tag="rc32")
            # block-diagonal masked lhsT for the abar matmuls:
            # rrbM[p, sc, n, col] = (col==n) * 1/rowsum_n[sc*128+p]
            rrbM = cp.tile([128, SCH, BL, BL], bf16, tag="rrbM")
            attnRows = cp.tile([BL, S], bf16, tag="attnRows")
            attnT = cp.tile([128, SCH, BL], bf16, tag="attnT")
            pooledT = cp.tile([128, 2, BL], f32, tag="pooledT")
            hT = cp.tile([128, BL], f32, tag="hT")
            lgT = cp.tile([16, BL], f32, tag="lgT")

            nc.sync.dma_start(wq_s[:], wq_d.ap())
            nc.sync.dma_start(eTs[0][:], et_d[0].ap())
            nc.sync.dma_start(wk_s[:], wk_d.ap())
            nc.sync.dma_start(pqt_s[:], pqt_d.ap())
            nc.sync.dma_start(pkt_s[:], pkt_d.ap())
            nc.sync.dma_start(wv_s[:], wv_d.ap())
            nc.sync.dma_start(eTs[1][:], et_d[1].ap())
            nc.sync.dma_start(eTs[2][:], et_d[2].ap())
            nc.sync.dma_start(pv_s[:], pv_d.ap())
            nc.sync.dma_start(wc1_s[:], wc1_d.ap())
            nc.sync.dma_start(bc1_s[:], bc1_d.ap())
            nc.sync.dma_start(wc2_s[:], wc2_d.ap())
            nc.sync.dma_start(bc2_s[:], bc2_d.ap())
            nc.sync.dma_start(eye_s[:], eye_d.ap())
            for n in range(3, BL):
                nc.sync.dma_start(eTs[n][:], et_d[n].ap())

            nc.gpsimd.memset(rrbM[:], 0.0)
            if STAGE < 7:
                nc.vector.memset(lgT[:], 0.0)

            # warm the PE p-state during the initial DMA wait: six dummy
            # matmuls on the (memset) rrbM tile keep the engine busy so real
            # matmuls start at full clock. Results are never read.
            warm = psB.tile([128, S], f32, tag="B")
            for _ in range(16):
                nc.tensor.matmul(
                    warm[0:BL, 0:BL * BL],
                    lhsT=rrbM[:, 0, 0, :],
                    rhs=rrbM[:, 0, :, :],
                    start=True,
                    stop=True,
                )

            psAB = psC.tile([128, S], f32, tag="AB")  # rows 0:BL used
            expTiles = [None] * BL

            def emit_abar(n):
                # abar row n (x S): sum_s exp[s, t] / rowsum[s], accumulated
                # into the shared psAB via the masked lhsT. One long
                # accumulation group across all batches.
                for sc in range(SCH):
                    nc.tensor.matmul(
                        psAB[0:BL, :],
                        lhsT=rrbM[:, sc, n, :],
                        rhs=expTiles[n][:, sc, :],
                        start=(n == 0 and sc == 0),
                        stop=(n == BL - 1 and sc == SCH - 1),
                        skip_group_check=True,
                    )

            for n in range(BL if STAGE >= 1 else 0):
                # ---- Q^T, K^T for batch n ----
                for m in range(2):
                    for w_s, pT_s, oT in ((wq_s, pqt_s, qT), (wk_s, pkt_s, kT)):
                        ps = psB.tile([128, S], f32, tag="B")
                        for k in range(2):
                            nc.tensor.matmul(
                                ps[:],
                                lhsT=w_s[:, k, m * 128:(m + 1) * 128],
                                rhs=eTs[n][:, k, 0:S],
                                start=(k == 0),
                                stop=(k == 1),
                            )
                        nc.vector.tensor_tensor(
                            out=oT[:, m, n * S:(n + 1) * S],
                            in0=ps[:],
                            in1=pT_s[:, m, :],
                            op=ADD,
                        )

                # ---- V = e @ Wv for batch n (pe/bv part folded into pooled) ----
                if STAGE >= 2:
                    for sc in range(SCH):
                        psv = psB.tile([128, D], f32, tag="B")
                        for k in range(2):
                            nc.tensor.matmul(
                                psv[:],
                                lhsT=eTs[n][:, k, sc * 128:(sc + 1) * 128],
                                rhs=wv_s[:, k, :],
                                start=(k == 0),
                                stop=(k == 1),
                            )
                        if sc % 2 == 0:
                            nc.vector.tensor_copy(
                                out=vS[:, n * SCH + sc, :], in_=psv[:]
                            )
                        else:
                            nc.scalar.copy(out=vS[:, n * SCH + sc, :], in_=psv[:])

                # ---- scores (s on partitions) + exp + rowsums ----
                if STAGE >= 3:
                    expT = wp.tile([128, SCH, S], bf16, tag="expT")
                    expTiles[n] = expT
                    for w in range(2):
                        ps = psA.tile([128, 2, S], f32, tag="A")
                        for i in range(2):
                            sc = 2 * w + i
                            for m in range(2):
                                nc.tensor.matmul(
                                    ps[:, i, :],
                                    lhsT=qT[:, m, n * S + sc * 128: n * S + (sc + 1) * 128],
                                    rhs=kT[:, m, n * S:(n + 1) * S],
                                    start=(m == 0),
                                    stop=(m == 1),
                                )
                        for i in range(2):
                            sc = 2 * w + i
                            nc.scalar.activation(
                                out=expT[:, sc, :],
                                in_=ps[:, i, :],
                                func=EXP,
                                scale=1.0 / 16.0,
                                accum_out=rsum[:, n, sc:sc + 1],
                            )
                    if n < BL - 1:
                        nc.vector.reciprocal(out=rc32[:, n, :], in_=rsum[:, n, :])
                        nc.vector.tensor_copy(out=rrbM[:, :, n, n], in_=rc32[:, n, :])
                    else:
                        # last batch: per-sc, so abar(7, sc) can start as soon
                        # as exp(7, sc) lands instead of after the whole chain
                        for sc in range(SCH):
                            nc.vector.reciprocal(
                                out=rc32[:, n, sc:sc + 1], in_=rsum[:, n, sc:sc + 1]
                            )
                            nc.vector.tensor_copy(
                                out=rrbM[:, sc, n, n:n + 1], in_=rc32[:, n, sc:sc + 1]
                            )

                    # software-pipelined by one batch: emit abar(n-1) here so
                    # PE never stalls waiting on this batch's exp/rowsum.
                    if STAGE >= 4 and n > 0:
                        emit_abar(n - 1)

            if STAGE >= 4:
                emit_abar(BL - 1)
                nc.scalar.copy(out=attnRows[0:BL, :], in_=psAB[0:BL, :])

            if STAGE >= 5:
                # transpose abar rows -> columns on PE: attnT[p, sc, b]
                for sc in range(SCH):
                    pst = psB.tile([128, BL], bf16, tag="B")
                    nc.tensor.transpose(
                        pst[:],
                        in_=attnRows[0:BL, sc * 128:(sc + 1) * 128],
                        identity=eye_s[:],
                    )
                    nc.scalar.copy(out=attnT[:, sc, :], in_=pst[:])

            if STAGE >= 6:
                # pooled^T (x S, folded into wc1): for each d-chunk, accumulate
                # all 8 batches into one [128, BL] psum (per-column groups).
                psp = psB.tile([128, 2, BL], f32, tag="B")
                for dch in range(2):
                    for n in range(BL):
                        for tc in range(SCH):
                            nc.tensor.matmul(
                                psp[:, dch, n:n + 1],
                                lhsT=vS[:, n * SCH + tc, dch * 128:(dch + 1) * 128],
                                rhs=attnT[:, tc, n:n + 1],
                                start=(tc == 0),
                                stop=False,
                                skip_group_check=True,
                            )
                            nc.tensor.matmul(
                                psp[:, dch, n:n + 1],
                                lhsT=pv_s[:, tc, dch * 128:(dch + 1) * 128],
                                rhs=attnT[:, tc, n:n + 1],
                                start=False,
                                stop=(tc == SCH - 1),
                                skip_group_check=True,
                            )
                nc.scalar.copy(out=pooledT[:], in_=psp[:])

            # ---- classifier ----
            if STAGE >= 7:
                hps = psB.tile([128, BL], f32, tag="B")
                for k in range(2):
                    nc.tensor.matmul(
                        hps[:, 0:BL],
                        lhsT=wc1_s[:, k, :],
                        rhs=pooledT[:, k, :],
                        start=(k == 0),
                        stop=(k == 1),
                    )
                nc.scalar.activation(
                    out=hT[:], in_=hps[:, 0:BL], func=RELU, bias=bc1_s[:]
                )

                lps = psB.tile([128, BL], f32, tag="B")
                nc.tensor.matmul(
                    lps[0:NCLS, 0:BL], lhsT=wc2_s[:], rhs=hT[:], start=True, stop=True
                )
                nc.scalar.activation(
                    out=lgT[:],
                    in_=lps[0:NCLS, 0:BL],
                    func=mybir.ActivationFunctionType.Identity,
                    bias=bc2_s[:],
                )
            nc.sync.dma_start(out_d.ap(), lgT[:])

    nc.compile()
    return nc


def prepare_in_maps(input_ids, emb, Wq, bq, Wk, bk, Wv, bv, Wc1, bc1, Wc2, bc2):
    pe = _pos_encoding().astype(np.float64)
    pQ = (pe @ Wq.astype(np.float64) + bq.astype(np.float64)).astype(np.float32)
    pK = (pe @ Wk.astype(np.float64) + bk.astype(np.float64)).astype(np.float32)
    pV = (pe @ Wv.astype(np.float64) + bv.astype(np.float64)).astype(np.float32)

    emb16 = emb.astype(ml_dtypes.bfloat16)

    def chunk_w(w):  # [D, D] -> [128, 2, D] bf16 with [p,k,j] = w[k*128+p, j]
        return np.ascontiguousarray(
            w.reshape(2, 128, D).transpose(1, 0, 2).astype(ml_dtypes.bfloat16)
        )

    wq16 = chunk_w(Wq)
    wk16 = chunk_w(Wk)
    wv16 = chunk_w(Wv)
    eye8 = np.eye(BL, dtype=ml_dtypes.bfloat16)

    def chunk_pT(p):  # [S, D] -> [128, 2, S] f32 with [p_,m,s] = p[s, m*128+p_]
        return np.ascontiguousarray(p.T.reshape(2, 128, S).transpose(1, 0, 2)).astype(
            np.float32
        )

    pqt = chunk_pT(pQ).astype(ml_dtypes.bfloat16)
    pkt = chunk_pT(pK).astype(ml_dtypes.bfloat16)
    # pv16[p, sc, d] = pV[sc*128+p, d]
    pv16 = np.ascontiguousarray(
        pV.reshape(SCH, 128, D).transpose(1, 0, 2)
    ).astype(ml_dtypes.bfloat16)
    # 1/S of the mean pooling is folded in here
    wc1 = np.ascontiguousarray(
        (Wc1 / np.float32(S)).reshape(2, 128, HID).transpose(1, 0, 2).astype(np.float32)
    )
    bc1c = np.ascontiguousarray(bc1.reshape(HID, 1).astype(np.float32))
    wc2 = np.ascontiguousarray(Wc2.astype(np.float32))
    bc2c = np.ascontiguousarray(bc2.reshape(NCLS, 1).astype(np.float32))

    in_maps = []
    for c in range(NCORES):
        m = dict(
            eye8=eye8,
            wq16=wq16,
            wk16=wk16,
            wv16=wv16,
            pqt=pqt,
            pkt=pkt,
            pv16=pv16,
            wc1=wc1,
            bc1c=bc1c,
            wc2=wc2,
            bc2c=bc2c,
        )
        for n in range(BL):
            e = emb16[input_ids[c * BL + n]]  # [S, D] bf16, host-side gather
            # et[p, k, s] = e[s, k*128+p]
            m[f"et{n}"] = np.ascontiguousarray(
                e.T.reshape(2, 128, S).transpose(1, 0, 2)
            )
        in_maps.append(m)
    return in_maps


_NC_CACHE = {}


def kernel(**inputs):
    inputs = {k: np.asarray(v) for k, v in inputs.items()}
    if "nc" not in _NC_CACHE:
        _NC_CACHE["nc"] = build_module()
    nc = _NC_CACHE["nc"]
    in_maps = prepare_in_maps(**inputs)
    res = run_bass_kernel_spmd(nc, in_maps, core_ids=list(range(NCORES)))
    out = np.empty((B, NCLS), dtype=np.float32)
    for c in range(NCORES):
        out[c * BL:(c + 1) * BL] = res.results[c]["lgt"].T
    return out
